# revision 70
# baseline (speedup 1.0000x reference)
"""Trainium2 Bass kernel for nn_MultiHeaded_4080218931880.

Multi-headed attention with the reference's *raw reshape* head split:
    q = from @ Wq + bq                      # (B, F, HD)
    q_r = q.reshape(B, H, D, F)             # raw row-major reshape
    score = einsum('bhdf,bhdt->bhft', q_r, k_r) * alpha
    probs = softmax(score + (1-mask)*NEG, axis=-1)
    out = einsum('bhft,bhdt->bhdf', probs, v_r).reshape(B, H*D, F)

Because the reshape is raw, head h only touches rows [2*D*h, 2*D*(h+1))
of the (F, HD) projection output, and the per-head (D, U) matrix is just
that row block flattened row-major: r[d', u] = proj[2d' + u//1024,
u%1024].  The 32 (b, h) pairs are fully independent: 4 pairs per core
over 8 cores.

Device program per core (all matmuls bf16 moving, fp32 PSUM):

Projections (pair j):
  q, k: x-block.T stationary layout (pre-transposed on host), W moving;
  alpha folded into k's PSUM eviction; a direct SBUF->SBUF DMA folds
  each evicted (128, 512) slice to the (64, 2, 512) head layout (the
  DMA iterates both access patterns row-major, which is exactly the raw
  reshape row 2d'+two -> partition d').
  v: computed TRANSPOSED (lhsT = W chunk, rhs = x.T chunk) so the
  (u, d') operand the context matmul needs comes straight out of PSUM --
  no PE transposes; an extra ones column rides along for the softmax
  denominator.

Attention (pair j), per u-chunk (128 u x 2048 f), software-pipelined
pd u-chunks deep so the PE never stalls behind ACT's exp:
  score^T via 4 matmuls (N=512) into fp32 PSUM halves (each half needs
  only one q column-slice; the host un-permutes f columns); exp on the
  ACT engine into a bf16 E tile.  Context is computed TRANSPOSED:
  ctx^T[f, d] accumulates with lhsT = E f-chunk (stationary, free in
  this cost regime) and the narrow (128, 64) v^T chunk moving -> out
  free size 64 instead of 512, halving PE time vs the untransposed
  form.  A 1-wide denominator matmul per f-chunk accumulates
  sum_u E[u, f] into its own PSUM bank.

Normalize: DVE reciprocal of the denominator + broadcast multiply into
bf16, DMA ctx^T (f-major) to DRAM; host transposes and upcasts.

Engine budget per core (timeline cost model): PE ~127us (projections
24.6k + score 32.8k + ctx 16.6k + den 0.3k rows/pair at 0.417ns/row),
ACT ~137us (exp is ACT-only on TRN2 hardware: 1024-col halves at
0.83ns/col + fixed access overhead), DVE ~19us, DMA ~32us.
"""

import numpy as np
from contextlib import ExitStack

import concourse.bass as bass
import concourse.bacc as bacc
import concourse.tile as tile
from concourse import mybir
from concourse.bass_utils import run_bass_kernel_spmd

BF16 = mybir.dt.bfloat16
F32 = mybir.dt.float32
NP_BF16 = mybir.dt.np(mybir.dt.bfloat16)

# Problem dims (hardcoded; harness runs kernel.py standalone).
B, F, T, C = 2, 2048, 2048, 1024
H, D = 16, 64
HD = H * D
ALPHA = 1.0 / np.sqrt(np.float32(D)).astype(np.float32)
NEG = -100000.0
N_CORES = 8
NPAIR = (B * H) // N_CORES  # 4 (b,h) pairs per core
P = 128

REAL_DIMS = dict(npair=NPAIR, c=C, hd=HD, d=D, f=F, t=T)


def build_program(has_mask=False, has_bias=True, dims=None,
                  evict_act="all", norm_pieces=2, last_pd=1,
                  main_pd=14, ebufs=17, nwarm=12):
    dm = dims or REAL_DIMS
    npair, c, hd, d, f, t = (
        dm["npair"], dm["c"], dm["hd"], dm["d"], dm["f"], dm["t"],
    )
    bh = 2 * d          # row-block height of x per (b,h) pair
    ncc = c // P        # contraction chunks for projections
    nch = t // P        # u-chunks for attention
    nfc = f // P        # f-chunks for the transposed context
    fh = f // 2

    nc = bacc.Bacc(None, target_bir_lowering=False, debug=True)
    # x and W arrive pre-permuted to their exact SBUF layouts (partition
    # dim outermost), so every load DMA is fully contiguous
    xfT = nc.declare_dram_parameter("xfT", [npair, P, ncc, bh], BF16, isOutput=False)
    xtT = nc.declare_dram_parameter("xtT", [npair, P, ncc, bh], BF16, isOutput=False)
    wq = nc.declare_dram_parameter("wq", [P, ncc, hd], BF16, isOutput=False)
    wk = nc.declare_dram_parameter("wk", [P, ncc, hd], BF16, isOutput=False)
    wv = nc.declare_dram_parameter("wv", [P, ncc, hd], BF16, isOutput=False)
    bq = nc.declare_dram_parameter("bq", [1, hd], BF16, isOutput=False)
    bk = nc.declare_dram_parameter("bk", [1, hd], BF16, isOutput=False)
    bv = nc.declare_dram_parameter("bv", [1, hd], BF16, isOutput=False)
    mbT = None
    if has_mask:
        # alpha lives in k, so the additive bias is exactly (1-mask).T*NEG
        mbT = nc.declare_dram_parameter("mbT", [t, f], BF16, isOutput=False)
    # ctx^T per pair: [p, fc, d] with f = fc*128 + p; host transposes.
    out_d = nc.declare_dram_parameter("out", [npair, P, nfc, d], BF16, isOutput=True)

    with tile.TileContext(nc) as tc, ExitStack() as ctx:
        const = ctx.enter_context(tc.tile_pool(name="const", bufs=1))
        wpool = ctx.enter_context(tc.tile_pool(name="wpool", bufs=1))
        xpool = ctx.enter_context(tc.tile_pool(name="xpool", bufs=2))
        blkpool = ctx.enter_context(tc.tile_pool(name="blkpool", bufs=2))
        rqk = ctx.enter_context(tc.tile_pool(name="rqk", bufs=2))
        vtp = ctx.enter_context(tc.tile_pool(name="vtp", bufs=2))
        epool = ctx.enter_context(tc.tile_pool(name="epool", bufs=ebufs))
        opool = ctx.enter_context(tc.tile_pool(name="opool", bufs=2))
        spool = ctx.enter_context(tc.tile_pool(name="spool", bufs=2))
        mpool = None
        if has_mask:
            mpool = ctx.enter_context(tc.tile_pool(name="mpool", bufs=3))

        # PSUM budget (8 banks of 2KB):
        #   pp_sc  2 bufs x (128,1024) f32 = 4 banks   score halves
        #   pp_cx  1 buf  x (128,16,64) f32 = 2 banks  ctx^T accumulator
        #   pp_dn  1 buf  x (128,16)    f32 = 1 bank   denominator acc
        #   pp_pj  1 buf  x (128,512)   f32 = 1 bank   projection slices
        pp_sc = ctx.enter_context(tc.tile_pool(name="pp_sc", bufs=2, space="PSUM"))
        pp_cx = ctx.enter_context(tc.tile_pool(name="pp_cx", bufs=1, space="PSUM"))
        pp_dn = ctx.enter_context(tc.tile_pool(name="pp_dn", bufs=1, space="PSUM"))
        pp_pj = ctx.enter_context(tc.tile_pool(name="pp_pj", bufs=1, space="PSUM"))

        if has_bias:
            ones_row = const.tile([1, P], BF16)
            nc.vector.memset(ones_row[:], 1.0)

        w_s, b_s = {}, {}

        def load_weight_quarter(name, wd, qs):
            if name not in w_s:
                w_s[name] = wpool.tile(
                    [P, ncc, hd], BF16, tag=f"w{name}", name=f"w{name}")
            nc.sync.dma_start(
                out=w_s[name][:, :, qs:qs + 256], in_=wd[:, :, qs:qs + 256])

        def load_weight_half(name, wd, hs):
            # one DMA per column-half so pair-0's first projection slice
            # waits for 1MB, not 2MB, and the DMA count stays low
            if name not in w_s:
                w_s[name] = wpool.tile(
                    [P, ncc, hd], BF16, tag=f"w{name}", name=f"w{name}")
            nc.sync.dma_start(
                out=w_s[name][:, :, hs:hs + 512], in_=wd[:, :, hs:hs + 512])

        def load_bias(name, bd):
            if has_bias:
                bt = wpool.tile([1, hd], BF16, tag=f"b{name}")
                nc.sync.dma_start(out=bt[:], in_=bd[:])
                b_s[name] = bt

        r_all = [{} for _ in range(npair)]
        vt_all = [None] * npair
        cx_hold = {}
        xf_all = [None] * npair
        xt_all = [None] * npair

        def load_x(j):
            xf_s = xpool.tile([P, ncc, bh], BF16, tag=f"xf{j}")
            nc.sync.dma_start(out=xf_s[:], in_=xfT[j])
            xt_s = xpool.tile([P, ncc, bh], BF16, tag=f"xt{j}")
            nc.sync.dma_start(out=xt_s[:], in_=xtT[j])
            xf_all[j], xt_all[j] = xf_s, xt_s

        def emit_proj_qk(j):
            """q/k projections for pair j (generator; x already loaded).
            Each 512-column PSUM slice is evicted to SBUF and immediately
            folded (128, 512) -> (64, 2, 512) by a SBUF->SBUF DMA (the DMA
            iterates both APs in row-major order, which realizes the raw
            reshape row 2d'+two -> partition d'), so the first score matmul
            only waits for the first k slice, not the whole projection."""
            xf_s, xt_s = xf_all[j], xt_all[j]
            blk_t, r_t = {}, {}
            for name in ("q", "k"):
                blk_t[name] = blkpool.tile(
                    [bh, hd], BF16, tag=f"blk{name}", name=f"blk{name}")
                r_t[name] = rqk.tile(
                    [d, 2, hd], BF16, tag=f"r{name}", name=f"r{name}")
                r_all[j][name] = r_t[name]
            # slice-major so pair-0's (q slice0, k slice0) complete before
            # either weight's second column-half has even arrived.  Pair 0's
            # FIRST-slice evictions run on ACT (idle during the head, and
            # they precede every exp in ACT's in-order queue); everything
            # else on DVE (ACT is the steady-state bottleneck).
            slice_order = ((("k", xt_s), ("q", xf_s)),
                           (("q", xf_s), ("k", xt_s)))
            for si, hs in enumerate(range(0, hd, 512)):
                if evict_act == "qfirst":
                    # slice-0 evicts + q's slice-1 evict on ACT (they gate
                    # the exp stream); k's slice-1 evict on DVE so it does
                    # not block the first exps in ACT's in-order queue
                    on_act_k = j == 0 and hs == 0
                    on_act_q = j == 0
                else:
                    on_act_k = on_act_q = j == 0 and (
                        evict_act == "all" or
                        (evict_act == "first" and hs == 0))
                subs = (0,)
                sw = 512
                for name, x_s in slice_order[si]:
                    blk, r = blk_t[name], r_t[name]
                    pj = pp_pj.tile([bh, 512], F32, tag="pj")
                    for sub in subs:
                        a, b = hs + sub, hs + sub + sw
                        if has_bias:
                            nc.tensor.matmul(
                                pj[:, sub:sub + sw], ones_row[:, :bh],
                                b_s[name][:, a:b],
                                start=True, stop=False,
                            )
                        for kc in range(ncc):
                            nc.tensor.matmul(
                                pj[:, sub:sub + sw], x_s[:, kc, :],
                                w_s[name][:, kc, a:b],
                                start=(kc == 0 and not has_bias),
                                stop=(kc == ncc - 1),
                            )
                            if kc == 3:
                                yield
                        if name == "k":
                            # fold alpha into k so exp needs no input scale
                            if on_act_k:
                                nc.scalar.mul(
                                    blk[:, a:b], pj[:, sub:sub + sw],
                                    float(ALPHA))
                            else:
                                nc.vector.tensor_scalar_mul(
                                    blk[:, a:b], pj[:, sub:sub + sw],
                                    float(ALPHA))
                        elif on_act_q:
                            nc.scalar.copy(blk[:, a:b], pj[:, sub:sub + sw])
                        else:
                            nc.vector.tensor_copy(
                                blk[:, a:b], pj[:, sub:sub + sw])
                        if name == "k" and len(subs) > 1:
                            # fold each k quarter immediately: chunk 0 only
                            # needs k columns [0:128]
                            nc.sync.dma_start(
                                out=r[:, :, a:b], in_=blk[:, a:b])
                    if not (name == "k" and len(subs) > 1):
                        nc.sync.dma_start(
                            out=r[:, :, hs:hs + 512], in_=blk[:, hs:hs + 512])
                    yield

        def emit_proj_v(j):
            # ---- v: transposed orientation (lhsT = W chunk, rhs = x.T) ----
            # psum slot mc4 holds v_projT rows [(4mg+mc4)*128 + p], i.e.
            # pv[p, mc4, r] = v_proj[x-row r, hd-col (4mg+mc4)*128 + p].
            # vt[p, mc, two, d'] = v_projT[128mc + p, 2d' + two]; the ctx
            # moving operand for u-chunk tc is vt[:, tc%8, tc//8, :].
            xt_s = xt_all[j]
            vt = vtp.tile([P, ncc, 2, d + 1], BF16, tag="vt")
            for mg in range(2):
                pv = pp_pj.tile([P, 4 * P], F32, tag="pj")
                for mc4 in range(4):
                    mc = 4 * mg + mc4
                    sl = pv[:, mc4 * P:(mc4 + 1) * P]
                    if has_bias:
                        nc.tensor.matmul(
                            sl, b_s["v"][:, mc * P:(mc + 1) * P],
                            ones_row[:, :P],
                            start=(mc4 == 0), stop=False,
                        )
                    for kc in range(ncc):
                        nc.tensor.matmul(
                            sl, w_s["v"][:, kc, mc * P:(mc + 1) * P],
                            xt_s[:, kc, :],
                            start=(mc4 == 0 and kc == 0 and not has_bias),
                            stop=(mc4 == 3 and kc == ncc - 1),
                        )
                    yield
                nc.vector.tensor_copy(
                    vt[:, 4 * mg:4 * mg + 4, :, 0:d],
                    pv[:].rearrange("p (g dd two) -> p g two dd", g=4, two=2),
                )
                yield
            nc.vector.memset(vt[:, :, :, d:d + 1], 1.0)
            vt_all[j] = vt
            yield

        def emit_attn(j, pre_ctx=None, pd=4):
            """Attention for pair j, software-pipelined one u-chunk deep:
            score+exp for chunk tc is emitted before ctx for chunk tc-1, so
            the PE never sits behind ACT's exp of the chunk it just scored.
            Yields let the driver slot projection matmuls into the stream.
            pre_ctx is invoked right before the first ctx matmul so the
            driver can finish emitting this pair's v^T producers (PE is
            in-order: a ctx matmul waiting on v^T emitted later would
            deadlock)."""
            r_q, r_k = r_all[j]["q"], r_all[j]["k"]
            cx = pp_cx.tile([P, nfc, d], F32, tag="cx")
            dn = pp_dn.tile([P, nfc], F32, tag="dn")
            e_tiles = {}

            def score_half(tcb, hf):
                # psum half hf holds f-cols {two*1024 + hf*512 + n},
                # i.e. exactly q column-slice hf — so exp of half 0
                # never waits for q's second slice (host un-permutes)
                if tcb not in e_tiles:
                    e_tiles[tcb] = epool.tile(
                        [P, f], BF16, tag="exp", name="exp")
                e = e_tiles[tcb]
                ktw, kn = tcb // ncc, (tcb % ncc) * P
                ps = pp_sc.tile([P, fh], F32, tag="sc")
                for two in range(2):
                    nc.tensor.matmul(
                        ps[:, two * 512:(two + 1) * 512],
                        r_k[:, ktw, kn:kn + P],
                        r_q[:, two, hf * 512:(hf + 1) * 512],
                        start=True, stop=True,
                    )
                if has_mask:
                    mt = mpool.tile([P, fh], BF16, tag="mb")
                    nc.sync.dma_start(
                        out=mt[:],
                        in_=mbT[tcb * P:(tcb + 1) * P,
                                hf * fh:(hf + 1) * fh],
                    )
                    nc.vector.tensor_add(ps[:], ps[:], mt[:])
                nc.scalar.activation(
                    e[:, hf * fh:(hf + 1) * fh], ps[:],
                    mybir.ActivationFunctionType.Exp,
                )

            def score_exp(tcb):
                score_half(tcb, 0)
                score_half(tcb, 1)

            def ctx_mm(tcb):
                e = e_tiles.pop(tcb)
                vt = vt_all[j]
                two, mc = tcb // ncc, tcb % ncc
                for fc in range(nfc):
                    ech = e[:, fc * P:(fc + 1) * P]
                    nc.tensor.matmul(
                        cx[:, fc, :], ech, vt[:, mc, two, 0:d],
                        start=(tcb == 0 and fc % 8 == 0),
                        stop=(tcb == nch - 1 and fc % 8 == 7),
                    )
                    nc.tensor.matmul(
                        dn[:, fc:fc + 1], ech, vt[:, mc, two, d:d + 1],
                        start=(tcb == 0 and fc == 0),
                        stop=(tcb == nch - 1 and fc == nfc - 1),
                    )

            # prologue: pd chunks of score/exp lookahead before any ctx.
            # For pair 0, emit all A-halves before any B-half: the A-halves
            # only need the first q/k column-slices, so exp starts while
            # the second slices are still in flight.
            ab = min(pd, 4)
            if j == 0:
                for tcb in range(ab):
                    score_half(tcb, 0)
                    yield "hold"
                for tcb in range(ab):
                    score_half(tcb, 1)
                    yield
                for tcb in range(ab, pd):
                    score_exp(tcb)
                    yield
            else:
                for tcb in range(pd):
                    score_exp(tcb)
                    yield
            for tcb in range(pd, nch):
                score_exp(tcb)
                yield
                if tcb == pd and pre_ctx is not None:
                    pre_ctx()
                ctx_mm(tcb - pd)
                yield
            if pre_ctx is not None and nch <= pd:
                pre_ctx()
            for tcb in range(max(0, nch - pd), nch):
                ctx_mm(tcb)
                yield "drain"
            cx_hold[j] = (cx, dn)

        def emit_norm(j, pieces=None):
            """Normalize pair j's ctx^T accumulator and store it (bf16;
            host upcasts).  Done in two halves so the first DMA overlaps
            the second multiply — matters for the last pair's tail."""
            cx, dn = cx_hold[j]
            rcp = spool.tile([P, nfc], F32, tag="rcp")
            nc.vector.reciprocal(rcp[:], dn[:])
            yield
            o = opool.tile([P, nfc, d], BF16, tag="o")
            hn = nfc // (pieces or norm_pieces)
            for hs in range(0, nfc, hn):
                nc.vector.tensor_mul(
                    o[:, hs:hs + hn, :], cx[:, hs:hs + hn, :],
                    rcp[:, hs:hs + hn, None].broadcast_to([P, hn, d]),
                )
                yield
                nc.sync.dma_start(
                    out=out_d[j][:, hs:hs + hn, :], in_=o[:, hs:hs + hn, :])
                yield

        # software pipeline: pair j's attention interleaved (in program
        # order, hence in each engine's instruction stream) with later
        # pairs' projections and pair j-1's normalization.  DMA issue
        # order is chosen so nothing ahead of a needed transfer can stall
        # the in-order PE queue: x0 + wq + wk first (pair-0 q/k path),
        # then pair-0's bounces, then wv and the remaining x tiles.
        from collections import deque

        # PE p-state warmup: dummy K=1 matmuls keep the tensor engine
        # issuing from t~0 so the cost model's ramp (warm after 3us of
        # activity) is already at full clock when the first projection runs
        if nwarm:
            wua = const.tile([1, 16], BF16)
            nc.vector.memset(wua[:], 0.5)
            wub = const.tile([1, 512], BF16)
            nc.vector.memset(wub[:], 0.5)
            # rotate through the (idle) score-psum ring and alternate halves
            # so consecutive dummies share no buffer: a WAW chain would make
            # each one wait and reset the ramp tracker it exists to feed
            for i in range(nwarm):
                wup = pp_sc.tile([P, fh], F32, tag="sc", name="wup")
                half = (i % 2) * 512
                nc.tensor.matmul(
                    wup[0:16, half:half + 512], wua[:], wub[:],
                    start=True, stop=True)
        # DMA issue order matches the head critical chain: k path first
        # (its projection is emitted first), then q, then second halves
        xt_s0 = xpool.tile([P, ncc, bh], BF16, tag="xt0", name="xt0")
        nc.sync.dma_start(out=xt_s0[:], in_=xtT[0])
        xf_s0 = xpool.tile([P, ncc, bh], BF16, tag="xf0", name="xf0")
        nc.sync.dma_start(out=xf_s0[:], in_=xfT[0])
        xf_all[0], xt_all[0] = xf_s0, xt_s0
        load_weight_half("k", wk, 0)
        load_weight_half("q", wq, 0)
        load_bias("q", bq)
        load_bias("k", bk)
        load_weight_half("q", wq, 512)
        load_weight_half("k", wk, 512)
        for _ in emit_proj_qk(0):
            pass
        load_weight_half("v", wv, 0)
        load_weight_half("v", wv, 512)
        load_bias("v", bv)
        for jx in range(1, npair):
            load_x(jx)

        gens = deque()
        gens.append(("v", 0, emit_proj_v(0)))
        done_v = set()

        def pump(n=1):
            done = 0
            while gens and done < n:
                try:
                    next(gens[0][2])
                    done += 1
                except StopIteration:
                    kind, jj, _ = gens.popleft()
                    if kind == "v":
                        done_v.add(jj)
            return done > 0

        def drain_v(j):
            while j not in done_v and gens:
                pump()

        ng = None
        for j in range(npair):
            if j + 1 < npair:
                gens.append(("qk", j + 1, emit_proj_qk(j + 1)))
                gens.append(("v", j + 1, emit_proj_v(j + 1)))
            for tok in emit_attn(j, pre_ctx=lambda j=j: drain_v(j),
                                 pd=main_pd if j + 1 < npair else last_pd):
                if tok == "hold":
                    continue
                if ng is not None:
                    next(ng, None)
                pump(1)
            drain_v(j + 1) if j + 1 < npair else None
            if ng is not None:
                for _ in ng:
                    pass
            ng = emit_norm(j)
        for _ in ng:
            pass

    nc.finalize()
    return nc


_PROGRAM_CACHE = {}
TRACE = False
LAST_RESULTS = None


def _get_program(has_mask, has_bias):
    key = (has_mask, has_bias)
    if key not in _PROGRAM_CACHE:
        _PROGRAM_CACHE[key] = build_program(has_mask=has_mask, has_bias=has_bias)
    return _PROGRAM_CACHE[key]


def kernel(**inputs):
    from_tensor = np.asarray(inputs["from_tensor"], np.float32)
    to_tensor = np.asarray(inputs["to_tensor"], np.float32)
    mask = np.asarray(inputs["mask"], np.float32)

    def wprep(w):
        # (C, HD) -> (P, C//P, HD): the device SBUF layout, so the weight
        # chunk DMAs are fully contiguous
        w = np.asarray(w, np.float32).astype(NP_BF16)
        return np.ascontiguousarray(
            w.reshape(C // 128, 128, HD).transpose(1, 0, 2)
        )

    wq = wprep(inputs["Wq"])
    wk = wprep(inputs["Wk"])
    wv = wprep(inputs["Wv"])
    bqv = np.asarray(inputs["bq"], np.float32).astype(NP_BF16).reshape(1, HD)
    bkv = np.asarray(inputs["bk"], np.float32).astype(NP_BF16).reshape(1, HD)
    bvv = np.asarray(inputs["bv"], np.float32).astype(NP_BF16).reshape(1, HD)

    mb = (1.0 - mask) * NEG  # (B, F, T) additive mask bias
    has_mask = bool(np.any(mb != 0.0))
    has_bias = bool(
        np.any(inputs["bq"]) or np.any(inputs["bk"]) or np.any(inputs["bv"])
    )
    nc = _get_program(has_mask, has_bias)

    bh = 2 * D

    def xprep(x, p):
        # block (bh, C) -> transpose -> (P, C//P, bh) SBUF layout
        xb = x[p // H, (p % H) * bh:(p % H + 1) * bh, :].T.astype(NP_BF16)
        return np.ascontiguousarray(
            xb.reshape(C // 128, 128, bh).transpose(1, 0, 2)
        )

    in_maps = []
    for core in range(N_CORES):
        pairs = [4 * core + jj for jj in range(NPAIR)]
        b = pairs[0] // H
        xf = np.stack([xprep(from_tensor, p) for p in pairs])
        xt = np.stack([xprep(to_tensor, p) for p in pairs])
        m = {
            "xfT": xf, "xtT": xt,
            "wq": wq, "wk": wk, "wv": wv,
            "bq": bqv, "bk": bkv, "bv": bvv,
        }
        if has_mask:
            # device f-column layout is (hf, two, n): f = two*1024+hf*512+n
            mt = np.ascontiguousarray(mb[b].T).astype(NP_BF16)
            mt = mt.reshape(T, 2, 2, 512).transpose(0, 2, 1, 3).reshape(T, F)
            m["mbT"] = np.ascontiguousarray(mt)
        in_maps.append(m)

    res = run_bass_kernel_spmd(
        nc, in_maps, core_ids=list(range(N_CORES)), trace=TRACE
    )
    global LAST_RESULTS
    LAST_RESULTS = res

    out = np.empty((B, HD, F), np.float32)
    for core in range(N_CORES):
        # (npair, P, nfc, d) bf16; f = fc*128 + p
        o = np.asarray(res.results[core]["out"], np.float32)
        for jj in range(NPAIR):
            p = 4 * core + jj
            b, h = p // H, p % H
            # device column index is (hf, two, n); f = two*1024 + hf*512 + n
            blk = o[jj].transpose(2, 1, 0).reshape(D, 2, 2, 512)
            out[b, h * D:(h + 1) * D, :] = (
                blk.transpose(0, 2, 1, 3).reshape(D, F)
            )
    return out


# revision 72
# speedup vs baseline: 1.0003x; 1.0003x over previous
"""Trainium2 Bass kernel for nn_MultiHeaded_4080218931880.

Multi-headed attention with the reference's *raw reshape* head split:
    q = from @ Wq + bq                      # (B, F, HD)
    q_r = q.reshape(B, H, D, F)             # raw row-major reshape
    score = einsum('bhdf,bhdt->bhft', q_r, k_r) * alpha
    probs = softmax(score + (1-mask)*NEG, axis=-1)
    out = einsum('bhft,bhdt->bhdf', probs, v_r).reshape(B, H*D, F)

Because the reshape is raw, head h only touches rows [2*D*h, 2*D*(h+1))
of the (F, HD) projection output, and the per-head (D, U) matrix is just
that row block flattened row-major: r[d', u] = proj[2d' + u//1024,
u%1024].  The 32 (b, h) pairs are fully independent: 4 pairs per core
over 8 cores.

Device program per core (all matmuls bf16 moving, fp32 PSUM):

Projections (pair j):
  q, k: x-block.T stationary layout (pre-transposed on host), W moving;
  alpha folded into k's PSUM eviction; a direct SBUF->SBUF DMA folds
  each evicted (128, 512) slice to the (64, 2, 512) head layout (the
  DMA iterates both access patterns row-major, which is exactly the raw
  reshape row 2d'+two -> partition d').
  v: computed TRANSPOSED (lhsT = W chunk, rhs = x.T chunk) so the
  (u, d') operand the context matmul needs comes straight out of PSUM --
  no PE transposes; an extra ones column rides along for the softmax
  denominator.

Attention (pair j), per u-chunk (128 u x 2048 f), software-pipelined
pd u-chunks deep so the PE never stalls behind ACT's exp:
  score^T via 4 matmuls (N=512) into fp32 PSUM halves (each half needs
  only one q column-slice; the host un-permutes f columns); exp on the
  ACT engine into a bf16 E tile.  Context is computed TRANSPOSED:
  ctx^T[f, d] accumulates with lhsT = E f-chunk (stationary, free in
  this cost regime) and the narrow (128, 64) v^T chunk moving -> out
  free size 64 instead of 512, halving PE time vs the untransposed
  form.  A 1-wide denominator matmul per f-chunk accumulates
  sum_u E[u, f] into its own PSUM bank.

Normalize: DVE reciprocal of the denominator + broadcast multiply into
bf16, DMA ctx^T (f-major) to DRAM; host transposes and upcasts.

Engine budget per core (timeline cost model): PE ~127us (projections
24.6k + score 32.8k + ctx 16.6k + den 0.3k rows/pair at 0.417ns/row),
ACT ~137us (exp is ACT-only on TRN2 hardware: 1024-col halves at
0.83ns/col + fixed access overhead), DVE ~19us, DMA ~32us.
"""

import numpy as np
from contextlib import ExitStack

import concourse.bass as bass
import concourse.bacc as bacc
import concourse.tile as tile
from concourse import mybir
from concourse.bass_utils import run_bass_kernel_spmd

BF16 = mybir.dt.bfloat16
F32 = mybir.dt.float32
NP_BF16 = mybir.dt.np(mybir.dt.bfloat16)

# Problem dims (hardcoded; harness runs kernel.py standalone).
B, F, T, C = 2, 2048, 2048, 1024
H, D = 16, 64
HD = H * D
ALPHA = 1.0 / np.sqrt(np.float32(D)).astype(np.float32)
NEG = -100000.0
N_CORES = 8
NPAIR = (B * H) // N_CORES  # 4 (b,h) pairs per core
P = 128

REAL_DIMS = dict(npair=NPAIR, c=C, hd=HD, d=D, f=F, t=T)


def build_program(has_mask=False, has_bias=True, dims=None,
                  evict_act="all", norm_pieces=2, last_pd=1,
                  main_pd=14, ebufs=17, nwarm=10, abw=5):
    dm = dims or REAL_DIMS
    npair, c, hd, d, f, t = (
        dm["npair"], dm["c"], dm["hd"], dm["d"], dm["f"], dm["t"],
    )
    bh = 2 * d          # row-block height of x per (b,h) pair
    ncc = c // P        # contraction chunks for projections
    nch = t // P        # u-chunks for attention
    nfc = f // P        # f-chunks for the transposed context
    fh = f // 2

    nc = bacc.Bacc(None, target_bir_lowering=False, debug=True)
    # x and W arrive pre-permuted to their exact SBUF layouts (partition
    # dim outermost), so every load DMA is fully contiguous
    xfT = nc.declare_dram_parameter("xfT", [npair, P, ncc, bh], BF16, isOutput=False)
    xtT = nc.declare_dram_parameter("xtT", [npair, P, ncc, bh], BF16, isOutput=False)
    wq = nc.declare_dram_parameter("wq", [P, ncc, hd], BF16, isOutput=False)
    wk = nc.declare_dram_parameter("wk", [P, ncc, hd], BF16, isOutput=False)
    wv = nc.declare_dram_parameter("wv", [P, ncc, hd], BF16, isOutput=False)
    bq = nc.declare_dram_parameter("bq", [1, hd], BF16, isOutput=False)
    bk = nc.declare_dram_parameter("bk", [1, hd], BF16, isOutput=False)
    bv = nc.declare_dram_parameter("bv", [1, hd], BF16, isOutput=False)
    mbT = None
    if has_mask:
        # alpha lives in k, so the additive bias is exactly (1-mask).T*NEG
        mbT = nc.declare_dram_parameter("mbT", [t, f], BF16, isOutput=False)
    # ctx^T per pair: [p, fc, d] with f = fc*128 + p; host transposes.
    out_d = nc.declare_dram_parameter("out", [npair, P, nfc, d], BF16, isOutput=True)

    with tile.TileContext(nc) as tc, ExitStack() as ctx:
        const = ctx.enter_context(tc.tile_pool(name="const", bufs=1))
        wpool = ctx.enter_context(tc.tile_pool(name="wpool", bufs=1))
        xpool = ctx.enter_context(tc.tile_pool(name="xpool", bufs=2))
        blkpool = ctx.enter_context(tc.tile_pool(name="blkpool", bufs=2))
        rqk = ctx.enter_context(tc.tile_pool(name="rqk", bufs=2))
        vtp = ctx.enter_context(tc.tile_pool(name="vtp", bufs=2))
        epool = ctx.enter_context(tc.tile_pool(name="epool", bufs=ebufs))
        opool = ctx.enter_context(tc.tile_pool(name="opool", bufs=2))
        spool = ctx.enter_context(tc.tile_pool(name="spool", bufs=2))
        mpool = None
        if has_mask:
            mpool = ctx.enter_context(tc.tile_pool(name="mpool", bufs=3))

        # PSUM budget (8 banks of 2KB):
        #   pp_sc  2 bufs x (128,1024) f32 = 4 banks   score halves
        #   pp_cx  1 buf  x (128,16,64) f32 = 2 banks  ctx^T accumulator
        #   pp_dn  1 buf  x (128,16)    f32 = 1 bank   denominator acc
        #   pp_pj  1 buf  x (128,512)   f32 = 1 bank   projection slices
        pp_sc = ctx.enter_context(tc.tile_pool(name="pp_sc", bufs=2, space="PSUM"))
        pp_cx = ctx.enter_context(tc.tile_pool(name="pp_cx", bufs=1, space="PSUM"))
        pp_dn = ctx.enter_context(tc.tile_pool(name="pp_dn", bufs=1, space="PSUM"))
        pp_pj = ctx.enter_context(tc.tile_pool(name="pp_pj", bufs=1, space="PSUM"))

        if has_bias:
            ones_row = const.tile([1, P], BF16)
            nc.vector.memset(ones_row[:], 1.0)

        w_s, b_s = {}, {}

        def load_weight_quarter(name, wd, qs):
            if name not in w_s:
                w_s[name] = wpool.tile(
                    [P, ncc, hd], BF16, tag=f"w{name}", name=f"w{name}")
            nc.sync.dma_start(
                out=w_s[name][:, :, qs:qs + 256], in_=wd[:, :, qs:qs + 256])

        def load_weight_half(name, wd, hs):
            # one DMA per column-half so pair-0's first projection slice
            # waits for 1MB, not 2MB, and the DMA count stays low
            if name not in w_s:
                w_s[name] = wpool.tile(
                    [P, ncc, hd], BF16, tag=f"w{name}", name=f"w{name}")
            nc.sync.dma_start(
                out=w_s[name][:, :, hs:hs + 512], in_=wd[:, :, hs:hs + 512])

        def load_bias(name, bd):
            if has_bias:
                bt = wpool.tile([1, hd], BF16, tag=f"b{name}")
                nc.sync.dma_start(out=bt[:], in_=bd[:])
                b_s[name] = bt

        r_all = [{} for _ in range(npair)]
        vt_all = [None] * npair
        cx_hold = {}
        xf_all = [None] * npair
        xt_all = [None] * npair

        def load_x(j):
            xf_s = xpool.tile([P, ncc, bh], BF16, tag=f"xf{j}")
            nc.sync.dma_start(out=xf_s[:], in_=xfT[j])
            xt_s = xpool.tile([P, ncc, bh], BF16, tag=f"xt{j}")
            nc.sync.dma_start(out=xt_s[:], in_=xtT[j])
            xf_all[j], xt_all[j] = xf_s, xt_s

        def emit_proj_qk(j):
            """q/k projections for pair j (generator; x already loaded).
            Each 512-column PSUM slice is evicted to SBUF and immediately
            folded (128, 512) -> (64, 2, 512) by a SBUF->SBUF DMA (the DMA
            iterates both APs in row-major order, which realizes the raw
            reshape row 2d'+two -> partition d'), so the first score matmul
            only waits for the first k slice, not the whole projection."""
            xf_s, xt_s = xf_all[j], xt_all[j]
            blk_t, r_t = {}, {}
            for name in ("q", "k"):
                blk_t[name] = blkpool.tile(
                    [bh, hd], BF16, tag=f"blk{name}", name=f"blk{name}")
                r_t[name] = rqk.tile(
                    [d, 2, hd], BF16, tag=f"r{name}", name=f"r{name}")
                r_all[j][name] = r_t[name]
            # slice-major so pair-0's (q slice0, k slice0) complete before
            # either weight's second column-half has even arrived.  Pair 0's
            # FIRST-slice evictions run on ACT (idle during the head, and
            # they precede every exp in ACT's in-order queue); everything
            # else on DVE (ACT is the steady-state bottleneck).
            slice_order = ((("k", xt_s), ("q", xf_s)),
                           (("q", xf_s), ("k", xt_s)))
            for si, hs in enumerate(range(0, hd, 512)):
                if evict_act == "qfirst":
                    # slice-0 evicts + q's slice-1 evict on ACT (they gate
                    # the exp stream); k's slice-1 evict on DVE so it does
                    # not block the first exps in ACT's in-order queue
                    on_act_k = j == 0 and hs == 0
                    on_act_q = j == 0
                else:
                    on_act_k = on_act_q = j == 0 and (
                        evict_act == "all" or
                        (evict_act == "first" and hs == 0))
                subs = (0,)
                sw = 512
                for name, x_s in slice_order[si]:
                    blk, r = blk_t[name], r_t[name]
                    pj = pp_pj.tile([bh, 512], F32, tag="pj")
                    for sub in subs:
                        a, b = hs + sub, hs + sub + sw
                        if has_bias:
                            nc.tensor.matmul(
                                pj[:, sub:sub + sw], ones_row[:, :bh],
                                b_s[name][:, a:b],
                                start=True, stop=False,
                            )
                        for kc in range(ncc):
                            nc.tensor.matmul(
                                pj[:, sub:sub + sw], x_s[:, kc, :],
                                w_s[name][:, kc, a:b],
                                start=(kc == 0 and not has_bias),
                                stop=(kc == ncc - 1),
                            )
                            if kc == 3:
                                yield
                        if name == "k":
                            # fold alpha into k so exp needs no input scale
                            if on_act_k:
                                nc.scalar.mul(
                                    blk[:, a:b], pj[:, sub:sub + sw],
                                    float(ALPHA))
                            else:
                                nc.vector.tensor_scalar_mul(
                                    blk[:, a:b], pj[:, sub:sub + sw],
                                    float(ALPHA))
                        elif on_act_q:
                            nc.scalar.copy(blk[:, a:b], pj[:, sub:sub + sw])
                        else:
                            nc.vector.tensor_copy(
                                blk[:, a:b], pj[:, sub:sub + sw])
                        if name == "k" and len(subs) > 1:
                            # fold each k quarter immediately: chunk 0 only
                            # needs k columns [0:128]
                            nc.sync.dma_start(
                                out=r[:, :, a:b], in_=blk[:, a:b])
                    if not (name == "k" and len(subs) > 1):
                        nc.sync.dma_start(
                            out=r[:, :, hs:hs + 512], in_=blk[:, hs:hs + 512])
                    yield

        def emit_proj_v(j):
            # ---- v: transposed orientation (lhsT = W chunk, rhs = x.T) ----
            # psum slot mc4 holds v_projT rows [(4mg+mc4)*128 + p], i.e.
            # pv[p, mc4, r] = v_proj[x-row r, hd-col (4mg+mc4)*128 + p].
            # vt[p, mc, two, d'] = v_projT[128mc + p, 2d' + two]; the ctx
            # moving operand for u-chunk tc is vt[:, tc%8, tc//8, :].
            xt_s = xt_all[j]
            vt = vtp.tile([P, ncc, 2, d + 1], BF16, tag="vt")
            for mg in range(2):
                pv = pp_pj.tile([P, 4 * P], F32, tag="pj")
                for mc4 in range(4):
                    mc = 4 * mg + mc4
                    sl = pv[:, mc4 * P:(mc4 + 1) * P]
                    if has_bias:
                        nc.tensor.matmul(
                            sl, b_s["v"][:, mc * P:(mc + 1) * P],
                            ones_row[:, :P],
                            start=(mc4 == 0), stop=False,
                        )
                    for kc in range(ncc):
                        nc.tensor.matmul(
                            sl, w_s["v"][:, kc, mc * P:(mc + 1) * P],
                            xt_s[:, kc, :],
                            start=(mc4 == 0 and kc == 0 and not has_bias),
                            stop=(mc4 == 3 and kc == ncc - 1),
                        )
                    yield
                nc.vector.tensor_copy(
                    vt[:, 4 * mg:4 * mg + 4, :, 0:d],
                    pv[:].rearrange("p (g dd two) -> p g two dd", g=4, two=2),
                )
                yield
            nc.vector.memset(vt[:, :, :, d:d + 1], 1.0)
            vt_all[j] = vt
            yield

        def emit_attn(j, pre_ctx=None, pd=4):
            """Attention for pair j, software-pipelined one u-chunk deep:
            score+exp for chunk tc is emitted before ctx for chunk tc-1, so
            the PE never sits behind ACT's exp of the chunk it just scored.
            Yields let the driver slot projection matmuls into the stream.
            pre_ctx is invoked right before the first ctx matmul so the
            driver can finish emitting this pair's v^T producers (PE is
            in-order: a ctx matmul waiting on v^T emitted later would
            deadlock)."""
            r_q, r_k = r_all[j]["q"], r_all[j]["k"]
            cx = pp_cx.tile([P, nfc, d], F32, tag="cx")
            dn = pp_dn.tile([P, nfc], F32, tag="dn")
            e_tiles = {}

            def score_half(tcb, hf):
                # psum half hf holds f-cols {two*1024 + hf*512 + n},
                # i.e. exactly q column-slice hf — so exp of half 0
                # never waits for q's second slice (host un-permutes)
                if tcb not in e_tiles:
                    e_tiles[tcb] = epool.tile(
                        [P, f], BF16, tag="exp", name="exp")
                e = e_tiles[tcb]
                ktw, kn = tcb // ncc, (tcb % ncc) * P
                ps = pp_sc.tile([P, fh], F32, tag="sc")
                for two in range(2):
                    nc.tensor.matmul(
                        ps[:, two * 512:(two + 1) * 512],
                        r_k[:, ktw, kn:kn + P],
                        r_q[:, two, hf * 512:(hf + 1) * 512],
                        start=True, stop=True,
                    )
                if has_mask:
                    mt = mpool.tile([P, fh], BF16, tag="mb")
                    nc.sync.dma_start(
                        out=mt[:],
                        in_=mbT[tcb * P:(tcb + 1) * P,
                                hf * fh:(hf + 1) * fh],
                    )
                    nc.vector.tensor_add(ps[:], ps[:], mt[:])
                nc.scalar.activation(
                    e[:, hf * fh:(hf + 1) * fh], ps[:],
                    mybir.ActivationFunctionType.Exp,
                )

            def score_exp(tcb):
                score_half(tcb, 0)
                score_half(tcb, 1)

            def ctx_mm(tcb):
                e = e_tiles.pop(tcb)
                vt = vt_all[j]
                two, mc = tcb // ncc, tcb % ncc
                for fc in range(nfc):
                    ech = e[:, fc * P:(fc + 1) * P]
                    nc.tensor.matmul(
                        cx[:, fc, :], ech, vt[:, mc, two, 0:d],
                        start=(tcb == 0 and fc % 8 == 0),
                        stop=(tcb == nch - 1 and fc % 8 == 7),
                    )
                    nc.tensor.matmul(
                        dn[:, fc:fc + 1], ech, vt[:, mc, two, d:d + 1],
                        start=(tcb == 0 and fc == 0),
                        stop=(tcb == nch - 1 and fc == nfc - 1),
                    )

            # prologue: pd chunks of score/exp lookahead before any ctx.
            # For pair 0, emit all A-halves before any B-half: the A-halves
            # only need the first q/k column-slices, so exp starts while
            # the second slices are still in flight.
            ab = min(pd, abw)
            if j == 0:
                for tcb in range(ab):
                    score_half(tcb, 0)
                    yield "hold"
                for tcb in range(ab):
                    score_half(tcb, 1)
                    yield
                for tcb in range(ab, pd):
                    score_exp(tcb)
                    yield
            else:
                for tcb in range(pd):
                    score_exp(tcb)
                    yield
            for tcb in range(pd, nch):
                score_exp(tcb)
                yield
                if tcb == pd and pre_ctx is not None:
                    pre_ctx()
                ctx_mm(tcb - pd)
                yield
            if pre_ctx is not None and nch <= pd:
                pre_ctx()
            for tcb in range(max(0, nch - pd), nch):
                ctx_mm(tcb)
                yield "drain"
            cx_hold[j] = (cx, dn)

        def emit_norm(j, pieces=None):
            """Normalize pair j's ctx^T accumulator and store it (bf16;
            host upcasts).  Done in two halves so the first DMA overlaps
            the second multiply — matters for the last pair's tail."""
            cx, dn = cx_hold[j]
            rcp = spool.tile([P, nfc], F32, tag="rcp")
            nc.vector.reciprocal(rcp[:], dn[:])
            yield
            o = opool.tile([P, nfc, d], BF16, tag="o")
            hn = nfc // (pieces or norm_pieces)
            for hs in range(0, nfc, hn):
                nc.vector.tensor_mul(
                    o[:, hs:hs + hn, :], cx[:, hs:hs + hn, :],
                    rcp[:, hs:hs + hn, None].broadcast_to([P, hn, d]),
                )
                yield
                nc.sync.dma_start(
                    out=out_d[j][:, hs:hs + hn, :], in_=o[:, hs:hs + hn, :])
                yield

        # software pipeline: pair j's attention interleaved (in program
        # order, hence in each engine's instruction stream) with later
        # pairs' projections and pair j-1's normalization.  DMA issue
        # order is chosen so nothing ahead of a needed transfer can stall
        # the in-order PE queue: x0 + wq + wk first (pair-0 q/k path),
        # then pair-0's bounces, then wv and the remaining x tiles.
        from collections import deque

        # PE p-state warmup: dummy K=1 matmuls keep the tensor engine
        # issuing from t~0 so the cost model's ramp (warm after 3us of
        # activity) is already at full clock when the first projection runs
        if nwarm:
            wua = const.tile([1, 16], BF16)
            nc.vector.memset(wua[:], 0.5)
            wub = const.tile([1, 512], BF16)
            nc.vector.memset(wub[:], 0.5)
            # rotate through the (idle) score-psum ring and alternate halves
            # so consecutive dummies share no buffer: a WAW chain would make
            # each one wait and reset the ramp tracker it exists to feed
            for i in range(nwarm):
                wup = pp_sc.tile([P, fh], F32, tag="sc", name="wup")
                half = (i % 2) * 512
                nc.tensor.matmul(
                    wup[0:16, half:half + 512], wua[:], wub[:],
                    start=True, stop=True)
        # DMA issue order matches the head critical chain: k path first
        # (its projection is emitted first), then q, then second halves
        xt_s0 = xpool.tile([P, ncc, bh], BF16, tag="xt0", name="xt0")
        nc.sync.dma_start(out=xt_s0[:], in_=xtT[0])
        xf_s0 = xpool.tile([P, ncc, bh], BF16, tag="xf0", name="xf0")
        nc.sync.dma_start(out=xf_s0[:], in_=xfT[0])
        xf_all[0], xt_all[0] = xf_s0, xt_s0
        load_weight_half("k", wk, 0)
        load_weight_half("q", wq, 0)
        load_bias("q", bq)
        load_bias("k", bk)
        load_weight_half("q", wq, 512)
        load_weight_half("k", wk, 512)
        for _ in emit_proj_qk(0):
            pass
        load_weight_half("v", wv, 0)
        load_weight_half("v", wv, 512)
        load_bias("v", bv)
        for jx in range(1, npair):
            load_x(jx)

        gens = deque()
        gens.append(("v", 0, emit_proj_v(0)))
        done_v = set()

        def pump(n=1):
            done = 0
            while gens and done < n:
                try:
                    next(gens[0][2])
                    done += 1
                except StopIteration:
                    kind, jj, _ = gens.popleft()
                    if kind == "v":
                        done_v.add(jj)
            return done > 0

        def drain_v(j):
            while j not in done_v and gens:
                pump()

        ng = None
        for j in range(npair):
            if j + 1 < npair:
                gens.append(("qk", j + 1, emit_proj_qk(j + 1)))
                gens.append(("v", j + 1, emit_proj_v(j + 1)))
            for tok in emit_attn(j, pre_ctx=lambda j=j: drain_v(j),
                                 pd=main_pd if j + 1 < npair else last_pd):
                if tok == "hold":
                    continue
                if ng is not None:
                    next(ng, None)
                pump(1)
            drain_v(j + 1) if j + 1 < npair else None
            if ng is not None:
                for _ in ng:
                    pass
            ng = emit_norm(j)
        for _ in ng:
            pass

    nc.finalize()
    return nc


_PROGRAM_CACHE = {}
TRACE = False
LAST_RESULTS = None


def _get_program(has_mask, has_bias):
    key = (has_mask, has_bias)
    if key not in _PROGRAM_CACHE:
        _PROGRAM_CACHE[key] = build_program(has_mask=has_mask, has_bias=has_bias)
    return _PROGRAM_CACHE[key]


def kernel(**inputs):
    from_tensor = np.asarray(inputs["from_tensor"], np.float32)
    to_tensor = np.asarray(inputs["to_tensor"], np.float32)
    mask = np.asarray(inputs["mask"], np.float32)

    def wprep(w):
        # (C, HD) -> (P, C//P, HD): the device SBUF layout, so the weight
        # chunk DMAs are fully contiguous
        w = np.asarray(w, np.float32).astype(NP_BF16)
        return np.ascontiguousarray(
            w.reshape(C // 128, 128, HD).transpose(1, 0, 2)
        )

    wq = wprep(inputs["Wq"])
    wk = wprep(inputs["Wk"])
    wv = wprep(inputs["Wv"])
    bqv = np.asarray(inputs["bq"], np.float32).astype(NP_BF16).reshape(1, HD)
    bkv = np.asarray(inputs["bk"], np.float32).astype(NP_BF16).reshape(1, HD)
    bvv = np.asarray(inputs["bv"], np.float32).astype(NP_BF16).reshape(1, HD)

    mb = (1.0 - mask) * NEG  # (B, F, T) additive mask bias
    has_mask = bool(np.any(mb != 0.0))
    has_bias = bool(
        np.any(inputs["bq"]) or np.any(inputs["bk"]) or np.any(inputs["bv"])
    )
    nc = _get_program(has_mask, has_bias)

    bh = 2 * D

    def xprep(x, p):
        # block (bh, C) -> transpose -> (P, C//P, bh) SBUF layout
        xb = x[p // H, (p % H) * bh:(p % H + 1) * bh, :].T.astype(NP_BF16)
        return np.ascontiguousarray(
            xb.reshape(C // 128, 128, bh).transpose(1, 0, 2)
        )

    in_maps = []
    for core in range(N_CORES):
        pairs = [4 * core + jj for jj in range(NPAIR)]
        b = pairs[0] // H
        xf = np.stack([xprep(from_tensor, p) for p in pairs])
        xt = np.stack([xprep(to_tensor, p) for p in pairs])
        m = {
            "xfT": xf, "xtT": xt,
            "wq": wq, "wk": wk, "wv": wv,
            "bq": bqv, "bk": bkv, "bv": bvv,
        }
        if has_mask:
            # device f-column layout is (hf, two, n): f = two*1024+hf*512+n
            mt = np.ascontiguousarray(mb[b].T).astype(NP_BF16)
            mt = mt.reshape(T, 2, 2, 512).transpose(0, 2, 1, 3).reshape(T, F)
            m["mbT"] = np.ascontiguousarray(mt)
        in_maps.append(m)

    res = run_bass_kernel_spmd(
        nc, in_maps, core_ids=list(range(N_CORES)), trace=TRACE
    )
    global LAST_RESULTS
    LAST_RESULTS = res

    out = np.empty((B, HD, F), np.float32)
    for core in range(N_CORES):
        # (npair, P, nfc, d) bf16; f = fc*128 + p
        o = np.asarray(res.results[core]["out"], np.float32)
        for jj in range(NPAIR):
            p = 4 * core + jj
            b, h = p // H, p % H
            # device column index is (hf, two, n); f = two*1024 + hf*512 + n
            blk = o[jj].transpose(2, 1, 0).reshape(D, 2, 2, 512)
            out[b, h * D:(h + 1) * D, :] = (
                blk.transpose(0, 2, 1, 3).reshape(D, F)
            )
    return out


# revision 75
# speedup vs baseline: 1.0047x; 1.0044x over previous
"""Trainium2 Bass kernel for nn_MultiHeaded_4080218931880.

Multi-headed attention with the reference's *raw reshape* head split:
    q = from @ Wq + bq                      # (B, F, HD)
    q_r = q.reshape(B, H, D, F)             # raw row-major reshape
    score = einsum('bhdf,bhdt->bhft', q_r, k_r) * alpha
    probs = softmax(score + (1-mask)*NEG, axis=-1)
    out = einsum('bhft,bhdt->bhdf', probs, v_r).reshape(B, H*D, F)

Because the reshape is raw, head h only touches rows [2*D*h, 2*D*(h+1))
of the (F, HD) projection output, and the per-head (D, U) matrix is just
that row block flattened row-major: r[d', u] = proj[2d' + u//1024,
u%1024].  The 32 (b, h) pairs are fully independent: 4 pairs per core
over 8 cores.

Device program per core (all matmuls bf16 moving, fp32 PSUM):

Projections (pair j):
  q, k: x-block.T stationary layout (pre-transposed on host), W moving;
  alpha folded into k's PSUM eviction; a direct SBUF->SBUF DMA folds
  each evicted (128, 512) slice to the (64, 2, 512) head layout (the
  DMA iterates both access patterns row-major, which is exactly the raw
  reshape row 2d'+two -> partition d').
  v: computed TRANSPOSED (lhsT = W chunk, rhs = x.T chunk) so the
  (u, d') operand the context matmul needs comes straight out of PSUM --
  no PE transposes; an extra ones column rides along for the softmax
  denominator.

Attention (pair j), per u-chunk (128 u x 2048 f), software-pipelined
pd u-chunks deep so the PE never stalls behind ACT's exp:
  score^T via 4 matmuls (N=512) into fp32 PSUM halves (each half needs
  only one q column-slice; the host un-permutes f columns); exp on the
  ACT engine into a bf16 E tile.  Context is computed TRANSPOSED:
  ctx^T[f, d] accumulates with lhsT = E f-chunk (stationary, free in
  this cost regime) and the narrow (128, 64) v^T chunk moving -> out
  free size 64 instead of 512, halving PE time vs the untransposed
  form.  A 1-wide denominator matmul per f-chunk accumulates
  sum_u E[u, f] into its own PSUM bank.

Normalize: DVE reciprocal of the denominator + broadcast multiply into
bf16, DMA ctx^T (f-major) to DRAM; host transposes and upcasts.

Engine budget per core (timeline cost model): PE ~127us (projections
24.6k + score 32.8k + ctx 16.6k + den 0.3k rows/pair at 0.417ns/row),
ACT ~137us (exp is ACT-only on TRN2 hardware: 1024-col halves at
0.83ns/col + fixed access overhead), DVE ~19us, DMA ~32us.
"""

import numpy as np
from contextlib import ExitStack

import concourse.bass as bass
import concourse.bacc as bacc
import concourse.tile as tile
from concourse import mybir
from concourse.bass_utils import run_bass_kernel_spmd

BF16 = mybir.dt.bfloat16
F32 = mybir.dt.float32
NP_BF16 = mybir.dt.np(mybir.dt.bfloat16)

# Problem dims (hardcoded; harness runs kernel.py standalone).
B, F, T, C = 2, 2048, 2048, 1024
H, D = 16, 64
HD = H * D
ALPHA = 1.0 / np.sqrt(np.float32(D)).astype(np.float32)
NEG = -100000.0
N_CORES = 8
NPAIR = (B * H) // N_CORES  # 4 (b,h) pairs per core
P = 128

REAL_DIMS = dict(npair=NPAIR, c=C, hd=HD, d=D, f=F, t=T)


def build_program(has_mask=False, has_bias=True, dims=None,
                  evict_act="all", norm_pieces=2, last_pd=1,
                  main_pd=14, ebufs=17, nwarm=10, abw=4):
    dm = dims or REAL_DIMS
    npair, c, hd, d, f, t = (
        dm["npair"], dm["c"], dm["hd"], dm["d"], dm["f"], dm["t"],
    )
    bh = 2 * d          # row-block height of x per (b,h) pair
    ncc = c // P        # contraction chunks for projections
    nch = t // P        # u-chunks for attention
    nfc = f // P        # f-chunks for the transposed context
    fh = f // 2

    nc = bacc.Bacc(None, target_bir_lowering=False, debug=True)
    # x and W arrive pre-permuted to their exact SBUF layouts (partition
    # dim outermost), so every load DMA is fully contiguous
    xfT = nc.declare_dram_parameter("xfT", [npair, P, ncc, bh], BF16, isOutput=False)
    xtT = nc.declare_dram_parameter("xtT", [npair, P, ncc, bh], BF16, isOutput=False)
    wq = nc.declare_dram_parameter("wq", [P, ncc, hd], BF16, isOutput=False)
    wk = nc.declare_dram_parameter("wk", [P, ncc, hd], BF16, isOutput=False)
    wv = nc.declare_dram_parameter("wv", [P, ncc, hd], BF16, isOutput=False)
    bq = nc.declare_dram_parameter("bq", [1, hd], BF16, isOutput=False)
    bk = nc.declare_dram_parameter("bk", [1, hd], BF16, isOutput=False)
    bv = nc.declare_dram_parameter("bv", [1, hd], BF16, isOutput=False)
    mbT = None
    if has_mask:
        # alpha lives in k, so the additive bias is exactly (1-mask).T*NEG
        mbT = nc.declare_dram_parameter("mbT", [t, f], BF16, isOutput=False)
    # ctx^T per pair: [p, fc, d] with f = fc*128 + p; host transposes.
    out_d = nc.declare_dram_parameter("out", [npair, P, nfc, d], BF16, isOutput=True)

    with tile.TileContext(nc) as tc, ExitStack() as ctx:
        const = ctx.enter_context(tc.tile_pool(name="const", bufs=1))
        wpool = ctx.enter_context(tc.tile_pool(name="wpool", bufs=1))
        xpool = ctx.enter_context(tc.tile_pool(name="xpool", bufs=2))
        blkpool = ctx.enter_context(tc.tile_pool(name="blkpool", bufs=2))
        rqk = ctx.enter_context(tc.tile_pool(name="rqk", bufs=2))
        vtp = ctx.enter_context(tc.tile_pool(name="vtp", bufs=2))
        epool = ctx.enter_context(tc.tile_pool(name="epool", bufs=ebufs))
        opool = ctx.enter_context(tc.tile_pool(name="opool", bufs=2))
        spool = ctx.enter_context(tc.tile_pool(name="spool", bufs=2))
        mpool = None
        if has_mask:
            mpool = ctx.enter_context(tc.tile_pool(name="mpool", bufs=3))

        # PSUM budget (8 banks of 2KB):
        #   pp_sc  2 bufs x (128,1024) f32 = 4 banks   score halves
        #   pp_cx  1 buf  x (128,16,64) f32 = 2 banks  ctx^T accumulator
        #   pp_dn  1 buf  x (128,16)    f32 = 1 bank   denominator acc
        #   pp_pj  1 buf  x (128,512)   f32 = 1 bank   projection slices
        pp_sc = ctx.enter_context(tc.tile_pool(name="pp_sc", bufs=2, space="PSUM"))
        pp_cx = ctx.enter_context(tc.tile_pool(name="pp_cx", bufs=1, space="PSUM"))
        pp_dn = ctx.enter_context(tc.tile_pool(name="pp_dn", bufs=1, space="PSUM"))
        pp_pj = ctx.enter_context(tc.tile_pool(name="pp_pj", bufs=1, space="PSUM"))

        if has_bias:
            ones_row = const.tile([1, P], BF16)
            nc.vector.memset(ones_row[:], 1.0)

        w_s, b_s = {}, {}

        def load_weight_quarter(name, wd, qs):
            if name not in w_s:
                w_s[name] = wpool.tile(
                    [P, ncc, hd], BF16, tag=f"w{name}", name=f"w{name}")
            nc.sync.dma_start(
                out=w_s[name][:, :, qs:qs + 256], in_=wd[:, :, qs:qs + 256])

        def load_weight_half(name, wd, hs):
            # one DMA per column-half so pair-0's first projection slice
            # waits for 1MB, not 2MB, and the DMA count stays low
            if name not in w_s:
                w_s[name] = wpool.tile(
                    [P, ncc, hd], BF16, tag=f"w{name}", name=f"w{name}")
            nc.sync.dma_start(
                out=w_s[name][:, :, hs:hs + 512], in_=wd[:, :, hs:hs + 512])

        def load_bias(name, bd):
            if has_bias:
                bt = wpool.tile([1, hd], BF16, tag=f"b{name}")
                nc.sync.dma_start(out=bt[:], in_=bd[:])
                b_s[name] = bt

        r_all = [{} for _ in range(npair)]
        vt_all = [None] * npair
        cx_hold = {}
        xf_all = [None] * npair
        xt_all = [None] * npair

        def load_x(j):
            xf_s = xpool.tile([P, ncc, bh], BF16, tag=f"xf{j}")
            nc.sync.dma_start(out=xf_s[:], in_=xfT[j])
            xt_s = xpool.tile([P, ncc, bh], BF16, tag=f"xt{j}")
            nc.sync.dma_start(out=xt_s[:], in_=xtT[j])
            xf_all[j], xt_all[j] = xf_s, xt_s

        def emit_proj_qk(j):
            """q/k projections for pair j (generator; x already loaded).
            Each 512-column PSUM slice is evicted to SBUF and immediately
            folded (128, 512) -> (64, 2, 512) by a SBUF->SBUF DMA (the DMA
            iterates both APs in row-major order, which realizes the raw
            reshape row 2d'+two -> partition d'), so the first score matmul
            only waits for the first k slice, not the whole projection."""
            xf_s, xt_s = xf_all[j], xt_all[j]
            blk_t, r_t = {}, {}
            for name in ("q", "k"):
                blk_t[name] = blkpool.tile(
                    [bh, hd], BF16, tag=f"blk{name}", name=f"blk{name}")
                r_t[name] = rqk.tile(
                    [d, 2, hd], BF16, tag=f"r{name}", name=f"r{name}")
                r_all[j][name] = r_t[name]
            # slice-major so pair-0's (q slice0, k slice0) complete before
            # either weight's second column-half has even arrived.  Pair 0's
            # FIRST-slice evictions run on ACT (idle during the head, and
            # they precede every exp in ACT's in-order queue); everything
            # else on DVE (ACT is the steady-state bottleneck).
            slice_order = ((("k", xt_s), ("q", xf_s)),
                           (("q", xf_s), ("k", xt_s)))
            for si, hs in enumerate(range(0, hd, 512)):
                if evict_act == "qfirst":
                    # slice-0 evicts + q's slice-1 evict on ACT (they gate
                    # the exp stream); k's slice-1 evict on DVE so it does
                    # not block the first exps in ACT's in-order queue
                    on_act_k = j == 0 and hs == 0
                    on_act_q = j == 0
                else:
                    on_act_k = on_act_q = j == 0 and (
                        evict_act == "all" or
                        (evict_act == "first" and hs == 0))
                subs = (0,)
                sw = 512
                for name, x_s in slice_order[si]:
                    blk, r = blk_t[name], r_t[name]
                    pj = pp_pj.tile([bh, 512], F32, tag="pj")
                    for sub in subs:
                        a, b = hs + sub, hs + sub + sw
                        if has_bias:
                            nc.tensor.matmul(
                                pj[:, sub:sub + sw], ones_row[:, :bh],
                                b_s[name][:, a:b],
                                start=True, stop=False,
                            )
                        for kc in range(ncc):
                            nc.tensor.matmul(
                                pj[:, sub:sub + sw], x_s[:, kc, :],
                                w_s[name][:, kc, a:b],
                                start=(kc == 0 and not has_bias),
                                stop=(kc == ncc - 1),
                            )
                            if kc == 3:
                                yield
                        if name == "k":
                            # fold alpha into k so exp needs no input scale
                            if on_act_k:
                                nc.scalar.mul(
                                    blk[:, a:b], pj[:, sub:sub + sw],
                                    float(ALPHA))
                            else:
                                nc.vector.tensor_scalar_mul(
                                    blk[:, a:b], pj[:, sub:sub + sw],
                                    float(ALPHA))
                        elif on_act_q:
                            nc.scalar.copy(blk[:, a:b], pj[:, sub:sub + sw])
                        else:
                            nc.vector.tensor_copy(
                                blk[:, a:b], pj[:, sub:sub + sw])
                        if name == "k" and len(subs) > 1:
                            # fold each k quarter immediately: chunk 0 only
                            # needs k columns [0:128]
                            nc.sync.dma_start(
                                out=r[:, :, a:b], in_=blk[:, a:b])
                    if not (name == "k" and len(subs) > 1):
                        nc.sync.dma_start(
                            out=r[:, :, hs:hs + 512], in_=blk[:, hs:hs + 512])
                    yield

        def emit_proj_v(j):
            # ---- v: transposed orientation (lhsT = W chunk, rhs = x.T) ----
            # psum slot mc4 holds v_projT rows [(4mg+mc4)*128 + p], i.e.
            # pv[p, mc4, r] = v_proj[x-row r, hd-col (4mg+mc4)*128 + p].
            # vt[p, mc, two, d'] = v_projT[128mc + p, 2d' + two]; the ctx
            # moving operand for u-chunk tc is vt[:, tc%8, tc//8, :].
            xt_s = xt_all[j]
            vt = vtp.tile([P, ncc, 2, d + 1], BF16, tag="vt")
            for mg in range(2):
                pv = pp_pj.tile([P, 4 * P], F32, tag="pj")
                for mc4 in range(4):
                    mc = 4 * mg + mc4
                    sl = pv[:, mc4 * P:(mc4 + 1) * P]
                    if has_bias:
                        nc.tensor.matmul(
                            sl, b_s["v"][:, mc * P:(mc + 1) * P],
                            ones_row[:, :P],
                            start=(mc4 == 0), stop=False,
                        )
                    for kc in range(ncc):
                        nc.tensor.matmul(
                            sl, w_s["v"][:, kc, mc * P:(mc + 1) * P],
                            xt_s[:, kc, :],
                            start=(mc4 == 0 and kc == 0 and not has_bias),
                            stop=(mc4 == 3 and kc == ncc - 1),
                        )
                    yield
                nc.vector.tensor_copy(
                    vt[:, 4 * mg:4 * mg + 4, :, 0:d],
                    pv[:].rearrange("p (g dd two) -> p g two dd", g=4, two=2),
                )
                yield
            nc.vector.memset(vt[:, :, :, d:d + 1], 1.0)
            vt_all[j] = vt
            yield

        def emit_attn(j, pre_ctx=None, pd=4):
            """Attention for pair j, software-pipelined one u-chunk deep:
            score+exp for chunk tc is emitted before ctx for chunk tc-1, so
            the PE never sits behind ACT's exp of the chunk it just scored.
            Yields let the driver slot projection matmuls into the stream.
            pre_ctx is invoked right before the first ctx matmul so the
            driver can finish emitting this pair's v^T producers (PE is
            in-order: a ctx matmul waiting on v^T emitted later would
            deadlock)."""
            r_q, r_k = r_all[j]["q"], r_all[j]["k"]
            cx = pp_cx.tile([P, nfc, d], F32, tag="cx")
            dn = pp_dn.tile([P, nfc], F32, tag="dn")
            e_tiles = {}

            def score_half(tcb, hf):
                # psum half hf holds f-cols {two*1024 + hf*512 + n},
                # i.e. exactly q column-slice hf — so exp of half 0
                # never waits for q's second slice (host un-permutes)
                if tcb not in e_tiles:
                    e_tiles[tcb] = epool.tile(
                        [P, f], BF16, tag="exp", name="exp")
                e = e_tiles[tcb]
                ktw, kn = tcb // ncc, (tcb % ncc) * P
                ps = pp_sc.tile([P, fh], F32, tag="sc")
                for two in range(2):
                    nc.tensor.matmul(
                        ps[:, two * 512:(two + 1) * 512],
                        r_k[:, ktw, kn:kn + P],
                        r_q[:, two, hf * 512:(hf + 1) * 512],
                        start=True, stop=True,
                    )
                if has_mask:
                    mt = mpool.tile([P, fh], BF16, tag="mb")
                    nc.sync.dma_start(
                        out=mt[:],
                        in_=mbT[tcb * P:(tcb + 1) * P,
                                hf * fh:(hf + 1) * fh],
                    )
                    nc.vector.tensor_add(ps[:], ps[:], mt[:])
                nc.scalar.activation(
                    e[:, hf * fh:(hf + 1) * fh], ps[:],
                    mybir.ActivationFunctionType.Exp,
                )

            def score_exp(tcb):
                score_half(tcb, 0)
                score_half(tcb, 1)

            def ctx_mm(tcb):
                e = e_tiles.pop(tcb)
                vt = vt_all[j]
                two, mc = tcb // ncc, tcb % ncc
                for fc in range(nfc):
                    ech = e[:, fc * P:(fc + 1) * P]
                    nc.tensor.matmul(
                        cx[:, fc, :], ech, vt[:, mc, two, 0:d],
                        start=(tcb == 0 and fc % 8 == 0),
                        stop=(tcb == nch - 1 and fc % 8 == 7),
                    )
                    nc.tensor.matmul(
                        dn[:, fc:fc + 1], ech, vt[:, mc, two, d:d + 1],
                        start=(tcb == 0 and fc == 0),
                        stop=(tcb == nch - 1 and fc == nfc - 1),
                    )

            # prologue: pd chunks of score/exp lookahead before any ctx.
            # For pair 0, emit all A-halves before any B-half: the A-halves
            # only need the first q/k column-slices, so exp starts while
            # the second slices are still in flight.
            ab = min(pd, abw)
            if j == 0:
                for tcb in range(ab):
                    score_half(tcb, 0)
                    yield "hold"
                for tcb in range(ab):
                    score_half(tcb, 1)
                    yield
                for tcb in range(ab, pd):
                    score_exp(tcb)
                    yield
            else:
                for tcb in range(pd):
                    score_exp(tcb)
                    yield
            for tcb in range(pd, nch):
                score_exp(tcb)
                yield
                if tcb == pd and pre_ctx is not None:
                    pre_ctx()
                ctx_mm(tcb - pd)
                yield
            if pre_ctx is not None and nch <= pd:
                pre_ctx()
            for tcb in range(max(0, nch - pd), nch):
                ctx_mm(tcb)
                yield "drain"
            cx_hold[j] = (cx, dn)

        def emit_norm(j, pieces=None):
            """Normalize pair j's ctx^T accumulator and store it (bf16;
            host upcasts).  Done in two halves so the first DMA overlaps
            the second multiply — matters for the last pair's tail."""
            cx, dn = cx_hold[j]
            rcp = spool.tile([P, nfc], F32, tag="rcp")
            nc.vector.reciprocal(rcp[:], dn[:])
            yield
            o = opool.tile([P, nfc, d], BF16, tag="o")
            hn = nfc // (pieces or norm_pieces)
            for hs in range(0, nfc, hn):
                nc.vector.tensor_mul(
                    o[:, hs:hs + hn, :], cx[:, hs:hs + hn, :],
                    rcp[:, hs:hs + hn, None].broadcast_to([P, hn, d]),
                )
                yield
                nc.sync.dma_start(
                    out=out_d[j][:, hs:hs + hn, :], in_=o[:, hs:hs + hn, :])
                yield

        # software pipeline: pair j's attention interleaved (in program
        # order, hence in each engine's instruction stream) with later
        # pairs' projections and pair j-1's normalization.  DMA issue
        # order is chosen so nothing ahead of a needed transfer can stall
        # the in-order PE queue: x0 + wq + wk first (pair-0 q/k path),
        # then pair-0's bounces, then wv and the remaining x tiles.
        from collections import deque

        # PE p-state warmup: dummy K=1 matmuls keep the tensor engine
        # issuing from t~0 so the cost model's ramp (warm after 3us of
        # activity) is already at full clock when the first projection runs
        if nwarm:
            wua = const.tile([1, 16], BF16)
            nc.vector.memset(wua[:], 0.5)
            wub = const.tile([1, 512], BF16)
            nc.vector.memset(wub[:], 0.5)
            # rotate through the (idle) score-psum ring and alternate halves
            # so consecutive dummies share no buffer: a WAW chain would make
            # each one wait and reset the ramp tracker it exists to feed
            for i in range(nwarm):
                wup = pp_sc.tile([P, fh], F32, tag="sc", name="wup")
                half = (i % 2) * 512
                nc.tensor.matmul(
                    wup[0:16, half:half + 512], wua[:], wub[:],
                    start=True, stop=True)
        # DMA issue order matches the head critical chain: k path first
        # (its projection is emitted first), then q, then second halves
        xt_s0 = xpool.tile([P, ncc, bh], BF16, tag="xt0", name="xt0")
        nc.sync.dma_start(out=xt_s0[:], in_=xtT[0])
        xf_s0 = xpool.tile([P, ncc, bh], BF16, tag="xf0", name="xf0")
        nc.sync.dma_start(out=xf_s0[:], in_=xfT[0])
        xf_all[0], xt_all[0] = xf_s0, xt_s0
        load_weight_half("k", wk, 0)
        load_weight_half("q", wq, 0)
        load_bias("q", bq)
        load_bias("k", bk)
        # second halves as quarters: their transfers have no data deps and
        # otherwise monopolize the DMA device in 2.9us blocks right when
        # pair-0's fold DMAs become ready; quarter boundaries let the small
        # folds slot in between
        load_weight_quarter("q", wq, 512)
        load_weight_quarter("q", wq, 768)
        load_weight_quarter("k", wk, 512)
        load_weight_quarter("k", wk, 768)
        for _ in emit_proj_qk(0):
            pass
        for qs in (0, 256, 512, 768):
            load_weight_quarter("v", wv, qs)
        load_bias("v", bv)
        for jx in range(1, npair):
            load_x(jx)

        gens = deque()
        gens.append(("v", 0, emit_proj_v(0)))
        done_v = set()

        def pump(n=1):
            done = 0
            while gens and done < n:
                try:
                    next(gens[0][2])
                    done += 1
                except StopIteration:
                    kind, jj, _ = gens.popleft()
                    if kind == "v":
                        done_v.add(jj)
            return done > 0

        def drain_v(j):
            while j not in done_v and gens:
                pump()

        ng = None
        for j in range(npair):
            if j + 1 < npair:
                gens.append(("qk", j + 1, emit_proj_qk(j + 1)))
                gens.append(("v", j + 1, emit_proj_v(j + 1)))
            for tok in emit_attn(j, pre_ctx=lambda j=j: drain_v(j),
                                 pd=main_pd if j + 1 < npair else last_pd):
                if tok == "hold":
                    continue
                if ng is not None:
                    next(ng, None)
                pump(1)
            drain_v(j + 1) if j + 1 < npair else None
            if ng is not None:
                for _ in ng:
                    pass
            ng = emit_norm(j)
        for _ in ng:
            pass

    nc.finalize()
    return nc


_PROGRAM_CACHE = {}
TRACE = False
LAST_RESULTS = None


def _get_program(has_mask, has_bias):
    key = (has_mask, has_bias)
    if key not in _PROGRAM_CACHE:
        _PROGRAM_CACHE[key] = build_program(has_mask=has_mask, has_bias=has_bias)
    return _PROGRAM_CACHE[key]


def kernel(**inputs):
    from_tensor = np.asarray(inputs["from_tensor"], np.float32)
    to_tensor = np.asarray(inputs["to_tensor"], np.float32)
    mask = np.asarray(inputs["mask"], np.float32)

    def wprep(w):
        # (C, HD) -> (P, C//P, HD): the device SBUF layout, so the weight
        # chunk DMAs are fully contiguous
        w = np.asarray(w, np.float32).astype(NP_BF16)
        return np.ascontiguousarray(
            w.reshape(C // 128, 128, HD).transpose(1, 0, 2)
        )

    wq = wprep(inputs["Wq"])
    wk = wprep(inputs["Wk"])
    wv = wprep(inputs["Wv"])
    bqv = np.asarray(inputs["bq"], np.float32).astype(NP_BF16).reshape(1, HD)
    bkv = np.asarray(inputs["bk"], np.float32).astype(NP_BF16).reshape(1, HD)
    bvv = np.asarray(inputs["bv"], np.float32).astype(NP_BF16).reshape(1, HD)

    mb = (1.0 - mask) * NEG  # (B, F, T) additive mask bias
    has_mask = bool(np.any(mb != 0.0))
    has_bias = bool(
        np.any(inputs["bq"]) or np.any(inputs["bk"]) or np.any(inputs["bv"])
    )
    nc = _get_program(has_mask, has_bias)

    bh = 2 * D

    def xprep(x, p):
        # block (bh, C) -> transpose -> (P, C//P, bh) SBUF layout
        xb = x[p // H, (p % H) * bh:(p % H + 1) * bh, :].T.astype(NP_BF16)
        return np.ascontiguousarray(
            xb.reshape(C // 128, 128, bh).transpose(1, 0, 2)
        )

    in_maps = []
    for core in range(N_CORES):
        pairs = [4 * core + jj for jj in range(NPAIR)]
        b = pairs[0] // H
        xf = np.stack([xprep(from_tensor, p) for p in pairs])
        xt = np.stack([xprep(to_tensor, p) for p in pairs])
        m = {
            "xfT": xf, "xtT": xt,
            "wq": wq, "wk": wk, "wv": wv,
            "bq": bqv, "bk": bkv, "bv": bvv,
        }
        if has_mask:
            # device f-column layout is (hf, two, n): f = two*1024+hf*512+n
            mt = np.ascontiguousarray(mb[b].T).astype(NP_BF16)
            mt = mt.reshape(T, 2, 2, 512).transpose(0, 2, 1, 3).reshape(T, F)
            m["mbT"] = np.ascontiguousarray(mt)
        in_maps.append(m)

    res = run_bass_kernel_spmd(
        nc, in_maps, core_ids=list(range(N_CORES)), trace=TRACE
    )
    global LAST_RESULTS
    LAST_RESULTS = res

    out = np.empty((B, HD, F), np.float32)
    for core in range(N_CORES):
        # (npair, P, nfc, d) bf16; f = fc*128 + p
        o = np.asarray(res.results[core]["out"], np.float32)
        for jj in range(NPAIR):
            p = 4 * core + jj
            b, h = p // H, p % H
            # device column index is (hf, two, n); f = two*1024 + hf*512 + n
            blk = o[jj].transpose(2, 1, 0).reshape(D, 2, 2, 512)
            out[b, h * D:(h + 1) * D, :] = (
                blk.transpose(0, 2, 1, 3).reshape(D, F)
            )
    return out


# revision 76
# speedup vs baseline: 1.0056x; 1.0009x over previous
"""Trainium2 Bass kernel for nn_MultiHeaded_4080218931880.

Multi-headed attention with the reference's *raw reshape* head split:
    q = from @ Wq + bq                      # (B, F, HD)
    q_r = q.reshape(B, H, D, F)             # raw row-major reshape
    score = einsum('bhdf,bhdt->bhft', q_r, k_r) * alpha
    probs = softmax(score + (1-mask)*NEG, axis=-1)
    out = einsum('bhft,bhdt->bhdf', probs, v_r).reshape(B, H*D, F)

Because the reshape is raw, head h only touches rows [2*D*h, 2*D*(h+1))
of the (F, HD) projection output, and the per-head (D, U) matrix is just
that row block flattened row-major: r[d', u] = proj[2d' + u//1024,
u%1024].  The 32 (b, h) pairs are fully independent: 4 pairs per core
over 8 cores.

Device program per core (all matmuls bf16 moving, fp32 PSUM):

Projections (pair j):
  q, k: x-block.T stationary layout (pre-transposed on host), W moving;
  alpha folded into k's PSUM eviction; a direct SBUF->SBUF DMA folds
  each evicted (128, 512) slice to the (64, 2, 512) head layout (the
  DMA iterates both access patterns row-major, which is exactly the raw
  reshape row 2d'+two -> partition d').
  v: computed TRANSPOSED (lhsT = W chunk, rhs = x.T chunk) so the
  (u, d') operand the context matmul needs comes straight out of PSUM --
  no PE transposes; an extra ones column rides along for the softmax
  denominator.

Attention (pair j), per u-chunk (128 u x 2048 f), software-pipelined
pd u-chunks deep so the PE never stalls behind ACT's exp:
  score^T via 4 matmuls (N=512) into fp32 PSUM halves (each half needs
  only one q column-slice; the host un-permutes f columns); exp on the
  ACT engine into a bf16 E tile.  Context is computed TRANSPOSED:
  ctx^T[f, d] accumulates with lhsT = E f-chunk (stationary, free in
  this cost regime) and the narrow (128, 64) v^T chunk moving -> out
  free size 64 instead of 512, halving PE time vs the untransposed
  form.  A 1-wide denominator matmul per f-chunk accumulates
  sum_u E[u, f] into its own PSUM bank.

Normalize: DVE reciprocal of the denominator + broadcast multiply into
bf16, DMA ctx^T (f-major) to DRAM; host transposes and upcasts.

Engine budget per core (timeline cost model): PE ~127us (projections
24.6k + score 32.8k + ctx 16.6k + den 0.3k rows/pair at 0.417ns/row),
ACT ~137us (exp is ACT-only on TRN2 hardware: 1024-col halves at
0.83ns/col + fixed access overhead), DVE ~19us, DMA ~32us.
"""

import numpy as np
from contextlib import ExitStack

import concourse.bass as bass
import concourse.bacc as bacc
import concourse.tile as tile
from concourse import mybir
from concourse.bass_utils import run_bass_kernel_spmd

BF16 = mybir.dt.bfloat16
F32 = mybir.dt.float32
NP_BF16 = mybir.dt.np(mybir.dt.bfloat16)

# Problem dims (hardcoded; harness runs kernel.py standalone).
B, F, T, C = 2, 2048, 2048, 1024
H, D = 16, 64
HD = H * D
ALPHA = 1.0 / np.sqrt(np.float32(D)).astype(np.float32)
NEG = -100000.0
N_CORES = 8
NPAIR = (B * H) // N_CORES  # 4 (b,h) pairs per core
P = 128

REAL_DIMS = dict(npair=NPAIR, c=C, hd=HD, d=D, f=F, t=T)


def build_program(has_mask=False, has_bias=True, dims=None,
                  evict_act="all", norm_pieces=2, last_pd=1,
                  main_pd=14, ebufs=17, nwarm=10, abw=4):
    dm = dims or REAL_DIMS
    npair, c, hd, d, f, t = (
        dm["npair"], dm["c"], dm["hd"], dm["d"], dm["f"], dm["t"],
    )
    bh = 2 * d          # row-block height of x per (b,h) pair
    ncc = c // P        # contraction chunks for projections
    nch = t // P        # u-chunks for attention
    nfc = f // P        # f-chunks for the transposed context
    fh = f // 2

    nc = bacc.Bacc(None, target_bir_lowering=False, debug=True)
    # x and W arrive pre-permuted to their exact SBUF layouts (partition
    # dim outermost), so every load DMA is fully contiguous
    xfT = nc.declare_dram_parameter("xfT", [npair, P, ncc, bh], BF16, isOutput=False)
    xtT = nc.declare_dram_parameter("xtT", [npair, P, ncc, bh], BF16, isOutput=False)
    wq = nc.declare_dram_parameter("wq", [P, ncc, hd], BF16, isOutput=False)
    wk = nc.declare_dram_parameter("wk", [P, ncc, hd], BF16, isOutput=False)
    wv = nc.declare_dram_parameter("wv", [P, ncc, hd], BF16, isOutput=False)
    bq = nc.declare_dram_parameter("bq", [1, hd], BF16, isOutput=False)
    bk = nc.declare_dram_parameter("bk", [1, hd], BF16, isOutput=False)
    bv = nc.declare_dram_parameter("bv", [1, hd], BF16, isOutput=False)
    mbT = None
    if has_mask:
        # alpha lives in k, so the additive bias is exactly (1-mask).T*NEG
        mbT = nc.declare_dram_parameter("mbT", [t, f], BF16, isOutput=False)
    # ctx^T per pair: [p, fc, d] with f = fc*128 + p; host transposes.
    out_d = nc.declare_dram_parameter("out", [npair, P, nfc, d], BF16, isOutput=True)

    with tile.TileContext(nc) as tc, ExitStack() as ctx:
        const = ctx.enter_context(tc.tile_pool(name="const", bufs=1))
        wpool = ctx.enter_context(tc.tile_pool(name="wpool", bufs=1))
        xpool = ctx.enter_context(tc.tile_pool(name="xpool", bufs=2))
        blkpool = ctx.enter_context(tc.tile_pool(name="blkpool", bufs=2))
        rqk = ctx.enter_context(tc.tile_pool(name="rqk", bufs=2))
        vtp = ctx.enter_context(tc.tile_pool(name="vtp", bufs=2))
        epool = ctx.enter_context(tc.tile_pool(name="epool", bufs=ebufs))
        opool = ctx.enter_context(tc.tile_pool(name="opool", bufs=2))
        spool = ctx.enter_context(tc.tile_pool(name="spool", bufs=2))
        mpool = None
        if has_mask:
            mpool = ctx.enter_context(tc.tile_pool(name="mpool", bufs=3))

        # PSUM budget (8 banks of 2KB):
        #   pp_sc  2 bufs x (128,1024) f32 = 4 banks   score halves
        #   pp_cx  1 buf  x (128,16,64) f32 = 2 banks  ctx^T accumulator
        #   pp_dn  1 buf  x (128,16)    f32 = 1 bank   denominator acc
        #   pp_pj  1 buf  x (128,512)   f32 = 1 bank   projection slices
        pp_sc = ctx.enter_context(tc.tile_pool(name="pp_sc", bufs=2, space="PSUM"))
        pp_cx = ctx.enter_context(tc.tile_pool(name="pp_cx", bufs=1, space="PSUM"))
        pp_dn = ctx.enter_context(tc.tile_pool(name="pp_dn", bufs=1, space="PSUM"))
        pp_pj = ctx.enter_context(tc.tile_pool(name="pp_pj", bufs=1, space="PSUM"))

        if has_bias:
            ones_row = const.tile([1, P], BF16)
            nc.vector.memset(ones_row[:], 1.0)

        w_s, b_s = {}, {}

        def load_weight_quarter(name, wd, qs):
            if name not in w_s:
                w_s[name] = wpool.tile(
                    [P, ncc, hd], BF16, tag=f"w{name}", name=f"w{name}")
            nc.sync.dma_start(
                out=w_s[name][:, :, qs:qs + 256], in_=wd[:, :, qs:qs + 256])

        def load_weight_half(name, wd, hs):
            # one DMA per column-half so pair-0's first projection slice
            # waits for 1MB, not 2MB, and the DMA count stays low
            if name not in w_s:
                w_s[name] = wpool.tile(
                    [P, ncc, hd], BF16, tag=f"w{name}", name=f"w{name}")
            nc.sync.dma_start(
                out=w_s[name][:, :, hs:hs + 512], in_=wd[:, :, hs:hs + 512])

        def load_bias(name, bd):
            if has_bias:
                bt = wpool.tile([1, hd], BF16, tag=f"b{name}")
                nc.sync.dma_start(out=bt[:], in_=bd[:])
                b_s[name] = bt

        r_all = [{} for _ in range(npair)]
        vt_all = [None] * npair
        cx_hold = {}
        xf_all = [None] * npair
        xt_all = [None] * npair

        def load_x(j, split=False):
            xf_s = xpool.tile([P, ncc, bh], BF16, tag=f"xf{j}")
            xt_s = xpool.tile([P, ncc, bh], BF16, tag=f"xt{j}")
            if split:
                for h in (0, ncc // 2):
                    nc.sync.dma_start(
                        out=xf_s[:, h:h + ncc // 2, :],
                        in_=xfT[j][:, h:h + ncc // 2, :])
                    nc.sync.dma_start(
                        out=xt_s[:, h:h + ncc // 2, :],
                        in_=xtT[j][:, h:h + ncc // 2, :])
            else:
                nc.sync.dma_start(out=xf_s[:], in_=xfT[j])
                nc.sync.dma_start(out=xt_s[:], in_=xtT[j])
            xf_all[j], xt_all[j] = xf_s, xt_s

        def emit_proj_qk(j):
            """q/k projections for pair j (generator; x already loaded).
            Each 512-column PSUM slice is evicted to SBUF and immediately
            folded (128, 512) -> (64, 2, 512) by a SBUF->SBUF DMA (the DMA
            iterates both APs in row-major order, which realizes the raw
            reshape row 2d'+two -> partition d'), so the first score matmul
            only waits for the first k slice, not the whole projection."""
            xf_s, xt_s = xf_all[j], xt_all[j]
            blk_t, r_t = {}, {}
            for name in ("q", "k"):
                blk_t[name] = blkpool.tile(
                    [bh, hd], BF16, tag=f"blk{name}", name=f"blk{name}")
                r_t[name] = rqk.tile(
                    [d, 2, hd], BF16, tag=f"r{name}", name=f"r{name}")
                r_all[j][name] = r_t[name]
            # slice-major so pair-0's (q slice0, k slice0) complete before
            # either weight's second column-half has even arrived.  Pair 0's
            # FIRST-slice evictions run on ACT (idle during the head, and
            # they precede every exp in ACT's in-order queue); everything
            # else on DVE (ACT is the steady-state bottleneck).
            slice_order = ((("k", xt_s), ("q", xf_s)),
                           (("q", xf_s), ("k", xt_s)))
            for si, hs in enumerate(range(0, hd, 512)):
                if evict_act == "qfirst":
                    # slice-0 evicts + q's slice-1 evict on ACT (they gate
                    # the exp stream); k's slice-1 evict on DVE so it does
                    # not block the first exps in ACT's in-order queue
                    on_act_k = j == 0 and hs == 0
                    on_act_q = j == 0
                else:
                    on_act_k = on_act_q = j == 0 and (
                        evict_act == "all" or
                        (evict_act == "first" and hs == 0))
                subs = (0,)
                sw = 512
                for name, x_s in slice_order[si]:
                    blk, r = blk_t[name], r_t[name]
                    pj = pp_pj.tile([bh, 512], F32, tag="pj")
                    for sub in subs:
                        a, b = hs + sub, hs + sub + sw
                        if has_bias:
                            nc.tensor.matmul(
                                pj[:, sub:sub + sw], ones_row[:, :bh],
                                b_s[name][:, a:b],
                                start=True, stop=False,
                            )
                        for kc in range(ncc):
                            nc.tensor.matmul(
                                pj[:, sub:sub + sw], x_s[:, kc, :],
                                w_s[name][:, kc, a:b],
                                start=(kc == 0 and not has_bias),
                                stop=(kc == ncc - 1),
                            )
                            if kc == 3:
                                yield
                        if name == "k":
                            # fold alpha into k so exp needs no input scale
                            if on_act_k:
                                nc.scalar.mul(
                                    blk[:, a:b], pj[:, sub:sub + sw],
                                    float(ALPHA))
                            else:
                                nc.vector.tensor_scalar_mul(
                                    blk[:, a:b], pj[:, sub:sub + sw],
                                    float(ALPHA))
                        elif on_act_q:
                            nc.scalar.copy(blk[:, a:b], pj[:, sub:sub + sw])
                        else:
                            nc.vector.tensor_copy(
                                blk[:, a:b], pj[:, sub:sub + sw])
                        if name == "k" and len(subs) > 1:
                            # fold each k quarter immediately: chunk 0 only
                            # needs k columns [0:128]
                            nc.sync.dma_start(
                                out=r[:, :, a:b], in_=blk[:, a:b])
                    if not (name == "k" and len(subs) > 1):
                        nc.sync.dma_start(
                            out=r[:, :, hs:hs + 512], in_=blk[:, hs:hs + 512])
                    yield

        def emit_proj_v(j):
            # ---- v: transposed orientation (lhsT = W chunk, rhs = x.T) ----
            # psum slot mc4 holds v_projT rows [(4mg+mc4)*128 + p], i.e.
            # pv[p, mc4, r] = v_proj[x-row r, hd-col (4mg+mc4)*128 + p].
            # vt[p, mc, two, d'] = v_projT[128mc + p, 2d' + two]; the ctx
            # moving operand for u-chunk tc is vt[:, tc%8, tc//8, :].
            xt_s = xt_all[j]
            vt = vtp.tile([P, ncc, 2, d + 1], BF16, tag="vt")
            for mg in range(2):
                pv = pp_pj.tile([P, 4 * P], F32, tag="pj")
                for mc4 in range(4):
                    mc = 4 * mg + mc4
                    sl = pv[:, mc4 * P:(mc4 + 1) * P]
                    if has_bias:
                        nc.tensor.matmul(
                            sl, b_s["v"][:, mc * P:(mc + 1) * P],
                            ones_row[:, :P],
                            start=(mc4 == 0), stop=False,
                        )
                    for kc in range(ncc):
                        nc.tensor.matmul(
                            sl, w_s["v"][:, kc, mc * P:(mc + 1) * P],
                            xt_s[:, kc, :],
                            start=(mc4 == 0 and kc == 0 and not has_bias),
                            stop=(mc4 == 3 and kc == ncc - 1),
                        )
                    yield
                nc.vector.tensor_copy(
                    vt[:, 4 * mg:4 * mg + 4, :, 0:d],
                    pv[:].rearrange("p (g dd two) -> p g two dd", g=4, two=2),
                )
                yield
            nc.vector.memset(vt[:, :, :, d:d + 1], 1.0)
            vt_all[j] = vt
            yield

        def emit_attn(j, pre_ctx=None, pd=4):
            """Attention for pair j, software-pipelined one u-chunk deep:
            score+exp for chunk tc is emitted before ctx for chunk tc-1, so
            the PE never sits behind ACT's exp of the chunk it just scored.
            Yields let the driver slot projection matmuls into the stream.
            pre_ctx is invoked right before the first ctx matmul so the
            driver can finish emitting this pair's v^T producers (PE is
            in-order: a ctx matmul waiting on v^T emitted later would
            deadlock)."""
            r_q, r_k = r_all[j]["q"], r_all[j]["k"]
            cx = pp_cx.tile([P, nfc, d], F32, tag="cx")
            dn = pp_dn.tile([P, nfc], F32, tag="dn")
            e_tiles = {}

            def score_half(tcb, hf):
                # psum half hf holds f-cols {two*1024 + hf*512 + n},
                # i.e. exactly q column-slice hf — so exp of half 0
                # never waits for q's second slice (host un-permutes)
                if tcb not in e_tiles:
                    e_tiles[tcb] = epool.tile(
                        [P, f], BF16, tag="exp", name="exp")
                e = e_tiles[tcb]
                ktw, kn = tcb // ncc, (tcb % ncc) * P
                ps = pp_sc.tile([P, fh], F32, tag="sc")
                for two in range(2):
                    nc.tensor.matmul(
                        ps[:, two * 512:(two + 1) * 512],
                        r_k[:, ktw, kn:kn + P],
                        r_q[:, two, hf * 512:(hf + 1) * 512],
                        start=True, stop=True,
                    )
                if has_mask:
                    mt = mpool.tile([P, fh], BF16, tag="mb")
                    nc.sync.dma_start(
                        out=mt[:],
                        in_=mbT[tcb * P:(tcb + 1) * P,
                                hf * fh:(hf + 1) * fh],
                    )
                    nc.vector.tensor_add(ps[:], ps[:], mt[:])
                nc.scalar.activation(
                    e[:, hf * fh:(hf + 1) * fh], ps[:],
                    mybir.ActivationFunctionType.Exp,
                )

            def score_exp(tcb):
                score_half(tcb, 0)
                score_half(tcb, 1)

            def ctx_mm(tcb):
                e = e_tiles.pop(tcb)
                vt = vt_all[j]
                two, mc = tcb // ncc, tcb % ncc
                for fc in range(nfc):
                    ech = e[:, fc * P:(fc + 1) * P]
                    nc.tensor.matmul(
                        cx[:, fc, :], ech, vt[:, mc, two, 0:d],
                        start=(tcb == 0 and fc % 8 == 0),
                        stop=(tcb == nch - 1 and fc % 8 == 7),
                    )
                    nc.tensor.matmul(
                        dn[:, fc:fc + 1], ech, vt[:, mc, two, d:d + 1],
                        start=(tcb == 0 and fc == 0),
                        stop=(tcb == nch - 1 and fc == nfc - 1),
                    )

            # prologue: pd chunks of score/exp lookahead before any ctx.
            # For pair 0, emit all A-halves before any B-half: the A-halves
            # only need the first q/k column-slices, so exp starts while
            # the second slices are still in flight.
            ab = min(pd, abw)
            if j == 0:
                for tcb in range(ab):
                    score_half(tcb, 0)
                    yield "hold"
                for tcb in range(ab):
                    score_half(tcb, 1)
                    yield
                for tcb in range(ab, pd):
                    score_exp(tcb)
                    yield
            else:
                for tcb in range(pd):
                    score_exp(tcb)
                    yield
            for tcb in range(pd, nch):
                score_exp(tcb)
                yield
                if tcb == pd and pre_ctx is not None:
                    pre_ctx()
                ctx_mm(tcb - pd)
                yield
            if pre_ctx is not None and nch <= pd:
                pre_ctx()
            for tcb in range(max(0, nch - pd), nch):
                ctx_mm(tcb)
                yield "drain"
            cx_hold[j] = (cx, dn)

        def emit_norm(j, pieces=None):
            """Normalize pair j's ctx^T accumulator and store it (bf16;
            host upcasts).  Done in two halves so the first DMA overlaps
            the second multiply — matters for the last pair's tail."""
            cx, dn = cx_hold[j]
            rcp = spool.tile([P, nfc], F32, tag="rcp")
            nc.vector.reciprocal(rcp[:], dn[:])
            yield
            o = opool.tile([P, nfc, d], BF16, tag="o")
            hn = nfc // (pieces or norm_pieces)
            for hs in range(0, nfc, hn):
                nc.vector.tensor_mul(
                    o[:, hs:hs + hn, :], cx[:, hs:hs + hn, :],
                    rcp[:, hs:hs + hn, None].broadcast_to([P, hn, d]),
                )
                yield
                nc.sync.dma_start(
                    out=out_d[j][:, hs:hs + hn, :], in_=o[:, hs:hs + hn, :])
                yield

        # software pipeline: pair j's attention interleaved (in program
        # order, hence in each engine's instruction stream) with later
        # pairs' projections and pair j-1's normalization.  DMA issue
        # order is chosen so nothing ahead of a needed transfer can stall
        # the in-order PE queue: x0 + wq + wk first (pair-0 q/k path),
        # then pair-0's bounces, then wv and the remaining x tiles.
        from collections import deque

        # PE p-state warmup: dummy K=1 matmuls keep the tensor engine
        # issuing from t~0 so the cost model's ramp (warm after 3us of
        # activity) is already at full clock when the first projection runs
        if nwarm:
            wua = const.tile([1, 16], BF16)
            nc.vector.memset(wua[:], 0.5)
            wub = const.tile([1, 512], BF16)
            nc.vector.memset(wub[:], 0.5)
            # rotate through the (idle) score-psum ring and alternate halves
            # so consecutive dummies share no buffer: a WAW chain would make
            # each one wait and reset the ramp tracker it exists to feed
            for i in range(nwarm):
                wup = pp_sc.tile([P, fh], F32, tag="sc", name="wup")
                half = (i % 2) * 512
                nc.tensor.matmul(
                    wup[0:16, half:half + 512], wua[:], wub[:],
                    start=True, stop=True)
        # DMA issue order matches the head critical chain: k path first
        # (its projection is emitted first), then q, then second halves
        xt_s0 = xpool.tile([P, ncc, bh], BF16, tag="xt0", name="xt0")
        nc.sync.dma_start(out=xt_s0[:], in_=xtT[0])
        xf_s0 = xpool.tile([P, ncc, bh], BF16, tag="xf0", name="xf0")
        nc.sync.dma_start(out=xf_s0[:], in_=xfT[0])
        xf_all[0], xt_all[0] = xf_s0, xt_s0
        load_weight_half("k", wk, 0)
        load_weight_half("q", wq, 0)
        load_bias("q", bq)
        load_bias("k", bk)
        # second halves as quarters: their transfers have no data deps and
        # otherwise monopolize the DMA device in 2.9us blocks right when
        # pair-0's fold DMAs become ready; quarter boundaries let the small
        # folds slot in between
        load_weight_quarter("q", wq, 512)
        load_weight_quarter("q", wq, 768)
        load_weight_quarter("k", wk, 512)
        load_weight_quarter("k", wk, 768)
        for _ in emit_proj_qk(0):
            pass
        for qs in (0, 256, 512, 768):
            load_weight_quarter("v", wv, qs)
        load_bias("v", bv)
        for jx in range(1, npair):
            load_x(jx, split=True)

        gens = deque()
        gens.append(("v", 0, emit_proj_v(0)))
        done_v = set()

        def pump(n=1):
            done = 0
            while gens and done < n:
                try:
                    next(gens[0][2])
                    done += 1
                except StopIteration:
                    kind, jj, _ = gens.popleft()
                    if kind == "v":
                        done_v.add(jj)
            return done > 0

        def drain_v(j):
            while j not in done_v and gens:
                pump()

        ng = None
        for j in range(npair):
            if j + 1 < npair:
                gens.append(("qk", j + 1, emit_proj_qk(j + 1)))
                gens.append(("v", j + 1, emit_proj_v(j + 1)))
            for tok in emit_attn(j, pre_ctx=lambda j=j: drain_v(j),
                                 pd=main_pd if j + 1 < npair else last_pd):
                if tok == "hold":
                    continue
                if ng is not None:
                    next(ng, None)
                pump(1)
            drain_v(j + 1) if j + 1 < npair else None
            if ng is not None:
                for _ in ng:
                    pass
            ng = emit_norm(j)
        for _ in ng:
            pass

    nc.finalize()
    return nc


_PROGRAM_CACHE = {}
TRACE = False
LAST_RESULTS = None


def _get_program(has_mask, has_bias):
    key = (has_mask, has_bias)
    if key not in _PROGRAM_CACHE:
        _PROGRAM_CACHE[key] = build_program(has_mask=has_mask, has_bias=has_bias)
    return _PROGRAM_CACHE[key]


def kernel(**inputs):
    from_tensor = np.asarray(inputs["from_tensor"], np.float32)
    to_tensor = np.asarray(inputs["to_tensor"], np.float32)
    mask = np.asarray(inputs["mask"], np.float32)

    def wprep(w):
        # (C, HD) -> (P, C//P, HD): the device SBUF layout, so the weight
        # chunk DMAs are fully contiguous
        w = np.asarray(w, np.float32).astype(NP_BF16)
        return np.ascontiguousarray(
            w.reshape(C // 128, 128, HD).transpose(1, 0, 2)
        )

    wq = wprep(inputs["Wq"])
    wk = wprep(inputs["Wk"])
    wv = wprep(inputs["Wv"])
    bqv = np.asarray(inputs["bq"], np.float32).astype(NP_BF16).reshape(1, HD)
    bkv = np.asarray(inputs["bk"], np.float32).astype(NP_BF16).reshape(1, HD)
    bvv = np.asarray(inputs["bv"], np.float32).astype(NP_BF16).reshape(1, HD)

    mb = (1.0 - mask) * NEG  # (B, F, T) additive mask bias
    has_mask = bool(np.any(mb != 0.0))
    has_bias = bool(
        np.any(inputs["bq"]) or np.any(inputs["bk"]) or np.any(inputs["bv"])
    )
    nc = _get_program(has_mask, has_bias)

    bh = 2 * D

    def xprep(x, p):
        # block (bh, C) -> transpose -> (P, C//P, bh) SBUF layout
        xb = x[p // H, (p % H) * bh:(p % H + 1) * bh, :].T.astype(NP_BF16)
        return np.ascontiguousarray(
            xb.reshape(C // 128, 128, bh).transpose(1, 0, 2)
        )

    in_maps = []
    for core in range(N_CORES):
        pairs = [4 * core + jj for jj in range(NPAIR)]
        b = pairs[0] // H
        xf = np.stack([xprep(from_tensor, p) for p in pairs])
        xt = np.stack([xprep(to_tensor, p) for p in pairs])
        m = {
            "xfT": xf, "xtT": xt,
            "wq": wq, "wk": wk, "wv": wv,
            "bq": bqv, "bk": bkv, "bv": bvv,
        }
        if has_mask:
            # device f-column layout is (hf, two, n): f = two*1024+hf*512+n
            mt = np.ascontiguousarray(mb[b].T).astype(NP_BF16)
            mt = mt.reshape(T, 2, 2, 512).transpose(0, 2, 1, 3).reshape(T, F)
            m["mbT"] = np.ascontiguousarray(mt)
        in_maps.append(m)

    res = run_bass_kernel_spmd(
        nc, in_maps, core_ids=list(range(N_CORES)), trace=TRACE
    )
    global LAST_RESULTS
    LAST_RESULTS = res

    out = np.empty((B, HD, F), np.float32)
    for core in range(N_CORES):
        # (npair, P, nfc, d) bf16; f = fc*128 + p
        o = np.asarray(res.results[core]["out"], np.float32)
        for jj in range(NPAIR):
            p = 4 * core + jj
            b, h = p // H, p % H
            # device column index is (hf, two, n); f = two*1024 + hf*512 + n
            blk = o[jj].transpose(2, 1, 0).reshape(D, 2, 2, 512)
            out[b, h * D:(h + 1) * D, :] = (
                blk.transpose(0, 2, 1, 3).reshape(D, F)
            )
    return out
